# revision 60
# baseline (speedup 1.0000x reference)
"""AttentionPairBias Trainium2 kernel (8 NeuronCores, query-sharded).

Strategy:
  - Shard the 1024 query rows across 8 cores (128 rows each). Each core reads
    only its slice of the huge pair tensor (512MB/8 = 64MB f32 -> 32MB bf16).
  - Host folds both LayerNorm affine transforms into the projection weights,
    centers the pair->bias weights so the pair-LN mean correction is free, and
    converts the pair slice to bf16 (DMA halved, enables DMA-transpose loads).
  - Per-head bias constant (ln_p_b @ Wb) is dropped: constant per (l,h) row is
    softmax-invariant.
  - On device, pair tiles arrive TRANSPOSED ([p, k] layout) via the DMA xbar,
    so the bias matmul contracts p on the PE with the pair tile as the
    stationary operand, producing [k, h] tiles; LN stats (mean / E[x^2]) come
    from extra matmul columns against ones/128; variance -> rsqrt is done as
    exp(-0.5*ln(var+eps)) so the whole kernel uses one ACT table set.
  - Attention runs transposed: logits^T[k,l] per (head, ktile), probs = exp()
    with the key-mask folded into the ACT bias operand, attn@v uses probs as
    the moving operand with a fused ones-column producing the softmax
    denominator for free. Output is built transposed, feeding the final Wo
    matmul without any extra transpose.
"""

import os

os.environ.setdefault("MYCRO_LOCAL_CACHE", "1")
# Tile's subtile dependency tracker mishandles interleaved strided APs (e.g.
# the [p, (dc, l)] transposed-activation writes) and lets consumers run before
# all producers; whole-tile deps are correct and cost nothing here since the
# kernel's phases are naturally sequential.
os.environ["BY_DEFAULT_DISABLE_SUBTILE_DEPS"] = "1"

import numpy as np
import ml_dtypes

# bass_utils imports antenv.axon_hooks unguarded when tracing is requested
# (e.g. BASS_TRACE=1 in the environment); some images lack that submodule.
# Provide the graceful no-hook fallback instead of an ImportError.
try:
    import antenv.axon_hooks  # noqa: F401
except ImportError:
    import sys as _sys
    import types as _types

    try:
        import antenv as _antenv
        _m = _types.ModuleType("antenv.axon_hooks")
        _hook = [None]
        _m.set_axon_ntff_profile_hook = lambda h: _hook.__setitem__(0, h)
        _m.get_axon_ntff_profile_hook = lambda: _hook[0]
        _sys.modules["antenv.axon_hooks"] = _m
        _antenv.axon_hooks = _m
    except ImportError:
        pass

# Prefer the ACT table set that contains Exp, Ln AND Square so the whole
# kernel needs exactly one table load. With the default set ordering the
# chooser alternates between an Exp/Square set and an Ln set inside the main
# loop, inserting ~270 table loads (~2.7us each).
import concourse.hw_specs as _hw_specs

_orig_get_act_tables = _hw_specs.get_activation_tables

def _patched_get_act_tables(arch):
    # Keep dict ORDER intact (set ids are positional — walrus loads tables by
    # index), but make natural_log_exp_and_others the only set offering Exp,
    # Ln and Square so every activation in this kernel resolves to one set.
    tabs = _orig_get_act_tables(arch)
    pref = "natural_log_exp_and_others"
    if pref not in tabs:
        return tabs
    strip = tabs[pref]
    return {
        k: (v if k == pref else (v - strip)) for k, v in tabs.items()
    }

_hw_specs.get_activation_tables = _patched_get_act_tables

import concourse.bass as bass
import concourse.bacc as bacc
import concourse.mybir as mybir
from concourse.bass_utils import run_bass_kernel_spmd
from concourse.tile import TileContext

F32 = mybir.dt.float32
F32R = mybir.dt.float32r
BF16 = mybir.dt.bfloat16
AF = mybir.ActivationFunctionType
ALU = mybir.AluOpType
AX = mybir.AxisListType

B, L, D, P, H = 1, 1024, 512, 128, 16
DH = D // H          # 32
NC = 8               # cores
LQ = L // NC         # 128 query rows per core
KT = L // 128        # 8 key tiles
DC = D // 128        # 4 D chunks
EPS = 1e-5

_CACHED = {}
LAST_INFO = {}
DEBUG = False


def _build_bass(phases="ABC", loop_n=None, use_mask=False):
    nc = bacc.Bacc("TRN2", target_bir_lowering=False, debug=False)
    # pair arrives HOST-pre-transposed: tile lb holds [p, (ls, k)] for the 4
    # query rows lb*4..lb*4+3 — a fully linear 1MB DMA (8KB per partition
    # row). The on-device DMA-transpose path ran at ~220GB/s and paced the
    # whole front half of the kernel; linear loads run at full HBM rate.
    pair_t = nc.declare_dram_parameter("pair_t", [LQ // 2, 128, 2 * L], BF16,
                                       isOutput=False)
    single = nc.declare_dram_parameter("single", [L, D], BF16, isOutput=False)
    wq = nc.declare_dram_parameter("wq", [128, 4 * D], BF16, isOutput=False)
    wk = nc.declare_dram_parameter("wk", [128, 4 * D], BF16, isOutput=False)
    wv = nc.declare_dram_parameter("wv", [128, 4 * D], BF16, isOutput=False)
    wg = nc.declare_dram_parameter("wg", [128, 4 * H], BF16, isOutput=False)
    wo = nc.declare_dram_parameter("wo", [128, 4 * D], BF16, isOutput=False)
    wbc = nc.declare_dram_parameter("wbc", [128, 17], BF16, isOutput=False)
    bq = nc.declare_dram_parameter("bq", [128, 4], F32, isOutput=False)
    bk = nc.declare_dram_parameter("bk", [128, 4], F32, isOutput=False)
    bv = nc.declare_dram_parameter("bv", [128, D], BF16, isOutput=False)
    bgn = nc.declare_dram_parameter("bgn", [1, H], BF16, isOutput=False)
    maskb = nc.declare_dram_parameter("maskb", [128, KT], F32, isOutput=False)
    ident = nc.declare_dram_parameter("ident", [128, 128], F32, isOutput=False)
    out = nc.declare_dram_parameter("out", [LQ, D], F32, isOutput=True)
    if DEBUG:
        d_gate = nc.declare_dram_parameter("d_gate", [LQ, H], F32, isOutput=True)
        d_kTb = nc.declare_dram_parameter("d_kTb", [128, 4 * L], BF16, isOutput=True)
        d_qTb = nc.declare_dram_parameter("d_qTb", [128, 4 * LQ], BF16, isOutput=True)
        d_biasT = nc.declare_dram_parameter("d_biasT", [128, KT * LQ * H], BF16, isOutput=True)
        d_outN = nc.declare_dram_parameter("d_outN", [LQ, D], F32, isOutput=True)
        d_vsb = nc.declare_dram_parameter("d_vsb", [128, KT * H * 33], BF16, isOutput=True)
        d_sT = nc.declare_dram_parameter("d_sT", [128, 4 * L], BF16, isOutput=True)

    with TileContext(nc) as tc:
        with tc.tile_pool(name="persist", bufs=1) as PS:
            kTb = PS.tile([128, 4 * L], BF16)        # [dk%128, (mc, k)]
            qTb = PS.tile([128, 4 * LQ], BF16)       # [dq%128, (mc, l)]
            v_sb = PS.tile([128, KT * (H * 33)], BF16)  # per kt: 16h x (32 v | 1 one)
            biasT = PS.tile([128, KT * LQ * H], BF16)   # [k, (kt, l, h)]
            gate = PS.tile([LQ, H], F32)
            wbc_t = PS.tile([128, 17], BF16)
            maskb_t = PS.tile([128, KT], F32)
            # weights split per-dc chunk: consumers read per-dc slices anyway,
            # and 4 separate 128KB DMAs spread across 4 queues instead of one
            # 512KB transfer camping on a single queue (~23us)
            wo_t = PS.tile([128, 4 * D], BF16)
            # gated attn out, split per 4-head group so the final transpose +
            # Wo accumulation can start as soon as its group's heads finish
            # (whole-tile deps would otherwise stall them to the very end)
            outN_g = [PS.tile([LQ, 4 * DH], F32, name=f"outN{dc}") for dc in range(DC)]
            outg_g = [PS.tile([128, LQ], BF16, name=f"outg{dc}") for dc in range(DC)]
            out_f = PS.tile([LQ, D], F32)
            id_t = PS.tile([128, 128], F32)
            eps_c = PS.tile([128, 1], F32)

            # Only the loads phase A1/B1 need immediately are issued here;
            # everything else is deferred behind the head-critical x + pair
            # triggers (each dma_start costs ~650ns of serial Sync-queue time,
            # so trigger ORDER sets the pipeline ramp).
            nc.sync.dma_start(out=id_t[:, :], in_=ident[:, :])
            nc.sync.dma_start(out=wbc_t[:, :], in_=wbc[:, :])
            nc.vector.memset(eps_c[:, :], EPS)
            import contextlib
            _loop_cm = tc.For_i(0, loop_n, 1) if loop_n else contextlib.nullcontext()
            with (
                _loop_cm,
                tc.tile_pool(name="pairp", bufs=16) as PP,
                tc.tile_pool(name="paw", bufs=1) as WW,
                tc.tile_pool(name="pax", bufs=8) as PX,
            ):
                # The first few pair tiles are issued BEFORE everything else:
                # their ~22us single-queue latency gates phase B's start, while
                # phase A tolerates its inputs arriving a few us later.
                npair = LQ // 2 if ("B" in phases or "D" in phases) else 0
                pt_tiles = []

                def emit_pair_dma(lb):
                    pt2 = PP.tile([128, 2 * L], BF16, tag="pt2")
                    nc.sync.dma_start(out=pt2[:, :], in_=pair_t[lb])
                    pt_tiles.append(pt2)

                # x tiles first (phase A starts off x[0]), then the pair head
                x_tiles = []
                for lt in range(L // 128 if "A" in phases else 0):
                    x = PX.tile([128, D], BF16, tag="x")
                    nc.sync.dma_start(out=x[:, :], in_=single[lt * 128:(lt + 1) * 128, :])
                    x_tiles.append(x)
                NPRE = 6
                for lb in range(min(NPRE, npair)):
                    emit_pair_dma(lb)
                # weights: not needed until A2 / phase C
                wq_t = WW.tile([128, 4 * D], BF16)
                wk_t = WW.tile([128, 4 * D], BF16)
                wv_t = WW.tile([128, 4 * D], BF16)
                wg_t = WW.tile([128, 4 * H], BF16)
                bq_t = WW.tile([128, 4], F32)
                bk_t = WW.tile([128, 4], F32)
                bv_t = WW.tile([128, D], BF16)
                bg_t = WW.tile([1, H], BF16)
                ones_t = WW.tile([1, LQ], BF16)
                nc.sync.dma_start(out=wq_t[:, :], in_=wq[:, :])
                nc.sync.dma_start(out=wk_t[:, :], in_=wk[:, :])
                nc.sync.dma_start(out=wv_t[:, :], in_=wv[:, :])
                nc.sync.dma_start(out=wg_t[:, :], in_=wg[:, :])
                nc.sync.dma_start(out=bq_t[:, :], in_=bq[:, :])
                nc.sync.dma_start(out=bk_t[:, :], in_=bk[:, :])
                nc.sync.dma_start(out=bv_t[:, :], in_=bv[:, :])
                nc.sync.dma_start(out=bg_t[:, :], in_=bgn[:, :])
                nc.sync.dma_start(out=maskb_t[:, :], in_=maskb[:, :])
                nc.sync.dma_start(out=wo_t[:, :], in_=wo[:, :])
                nc.vector.memset(ones_t[:, :], 1.0)
                # Remaining pair loads (pre-transposed on host). Each dma_start
                # lands on ONE of the 16 queues (~22GB/s each), so tile size
                # sets the latency-to-first-tile: half-size 512KB tiles (2
                # query rows) arrive in ~22us, and a 16-deep pool covers the
                # bandwidth-delay product so the stream never starves.
                # (Partition-split sub-DMAs are NOT used: <128-partition
                # transfers lose AXI ports to the swizzle and run ~1.6x
                # slower. Compute-queue issue is also out: a pool-gated DMA
                # trigger would stall all compute behind it.)
                for lb in range(NPRE, npair):
                    emit_pair_dma(lb)

                if "D" in phases and "B" not in phases:
                    with tc.tile_pool(name="dsink", bufs=2) as DS:
                        for lb in range(LQ // 2):
                            snk = DS.tile([128, 1], BF16, tag="snk")
                            nc.vector.tensor_copy(snk[:, :], pt_tiles[lb][:, 0:1])
                # ---------------- Phase A1: LN(single) -> sT -----------------
                # Engine queues are in-order, so phase A's PE instructions
                # ahead of phase B's would delay B ~30us past its data. Emit
                # A1 (cheap transposes), then the first B tiles, then A2 (the
                # projection matmuls) under B's engine slack, then the rest
                # of B.
                sT = PS.tile([128, 4 * L], BF16)  # xhat^T: [D%128, (dc, l)]
                with (
                    tc.tile_pool(name="pa", bufs=2) as PA,
                    tc.tile_pool(name="pasm", bufs=3) as SM,
                    tc.tile_pool(name="paps", bufs=2, space="PSUM") as PSA,
                ):
                    for lt in range(L // 128 if "A" in phases else 0):
                        x = x_tiles[lt]          # bf16 (upconverted on read)
                        S = SM.tile([128, 1], F32, tag="S")
                        nc.vector.tensor_reduce(S[:, :], x[:, :], AX.X, ALU.add)
                        sq = PA.tile([128, D], F32, tag="sq")
                        Q = SM.tile([128, 1], F32, tag="Q")
                        nc.scalar.activation(sq[:, :], x[:, :], AF.Square, accum_out=Q[:, :])
                        m = SM.tile([128, 1], F32, tag="m")
                        nc.vector.tensor_scalar(m[:, :], S[:, :], 1.0 / D, None, ALU.mult)
                        m2 = SM.tile([128, 1], F32, tag="m2")
                        nc.vector.tensor_tensor(m2[:, :], m[:, :], m[:, :], ALU.mult)
                        q2 = SM.tile([128, 1], F32, tag="q2")
                        nc.vector.tensor_scalar(q2[:, :], Q[:, :], 1.0 / D, EPS, ALU.mult, ALU.add)
                        ve = SM.tile([128, 1], F32, tag="ve")
                        nc.vector.tensor_tensor(ve[:, :], q2[:, :], m2[:, :], ALU.subtract)
                        lv = SM.tile([128, 1], F32, tag="lv")
                        nc.scalar.activation(lv[:, :], ve[:, :], AF.Ln)
                        sinv = SM.tile([128, 1], F32, tag="si")
                        nc.scalar.activation(sinv[:, :], lv[:, :], AF.Exp, scale=-0.5)
                        xh = PA.tile([128, D], F32, tag="xh")
                        nc.vector.tensor_scalar(xh[:, :], x[:, :], m[:, :], sinv[:, :],
                                                ALU.subtract, ALU.mult)
                        pst = PSA.tile([128, D], F32, tag="pst")
                        for j in range(DC):
                            nc.tensor.transpose(pst[:, j * 128:(j + 1) * 128],
                                                xh[:, j * 128:(j + 1) * 128], id_t[:, :])
                        o_ap = sT[:, :].rearrange("p (dc n) -> p dc n", dc=DC)[:, :, lt * 128:(lt + 1) * 128]
                        i_ap = pst[:, :].rearrange("p (dc j) -> p dc j", dc=DC)
                        nc.vector.tensor_copy(o_ap, i_ap)

                def emit_a2():
                    # ------------ Phase A2: projections off sT ---------------
                    with (
                        tc.tile_pool(name="pa2sm", bufs=3) as SM,
                        tc.tile_pool(name="paps2", bufs=1, space="PSUM") as PSA,
                    ):
                        # kT (keys, transposed, bf16)
                        for mc in range(4 if "A" in phases else 0):
                            for nb in range(2):
                                ps = PSA.tile([128, 512], F32, tag="kv")
                                for dc in range(DC):
                                    nc.tensor.matmul(
                                        ps[:, :],
                                        wk_t[:, dc * D + mc * 128: dc * D + (mc + 1) * 128],
                                        sT[:, dc * L + nb * 512: dc * L + (nb + 1) * 512],
                                        start=(dc == 0), stop=(dc == DC - 1))
                                nc.vector.tensor_scalar(
                                    kTb[:, mc * L + nb * 512: mc * L + (nb + 1) * 512],
                                    ps[:, :], bk_t[:, mc:mc + 1], None, ALU.add)
                        # v (natural layout, h-interleaved with ones column)
                        for kt in range(KT if "A" in phases else 0):
                            ps = PSA.tile([128, 512], F32, tag="kv")
                            for dc in range(DC):
                                nc.tensor.matmul(
                                    ps[:, :],
                                    sT[:, dc * L + kt * 128: dc * L + (kt + 1) * 128],
                                    wv_t[:, dc * D:(dc + 1) * D],
                                    start=(dc == 0), stop=(dc == DC - 1))
                            o_ap = v_sb[:, kt * (H * 33):(kt + 1) * (H * 33)].rearrange(
                                "p (h x) -> p h x", h=H)[:, :, 0:32]
                            nc.vector.tensor_tensor(
                                o_ap, ps[:, :].rearrange("p (h x) -> p h x", h=H),
                                bv_t[:, :].rearrange("p (h x) -> p h x", h=H), ALU.add)
                        # qT for own 128 rows (device rows 0..127 via host roll)
                        for mc in range(4 if "A" in phases else 0):
                            ps = PSA.tile([128, LQ], F32, tag="q")
                            for dc in range(DC):
                                nc.tensor.matmul(
                                    ps[:, :],
                                    wq_t[:, dc * D + mc * 128: dc * D + (mc + 1) * 128],
                                    sT[:, dc * L: dc * L + LQ],
                                    start=(dc == 0), stop=(dc == DC - 1))
                            nc.vector.tensor_scalar(
                                qTb[:, mc * LQ:(mc + 1) * LQ], ps[:, :],
                                bq_t[:, mc:mc + 1], None, ALU.add)
                        # gate = sigmoid(xhat @ Wg + bg) = 1/(1+exp(-x-bg))
                        if "A" not in phases:
                            nc.vector.memset(gate[:, :], 0.5)
                        psg = PSA.tile([LQ, H], F32, tag="g", name="psg") if "A" in phases else None
                        for dc in range(DC if "A" in phases else 0):
                            nc.tensor.matmul(
                                psg[:, :],
                                sT[:, dc * L: dc * L + LQ],
                                wg_t[:, dc * H:(dc + 1) * H],
                                start=(dc == 0), stop=False)
                        if "A" in phases:
                            nc.tensor.matmul(psg[:, :], ones_t[:, :], bg_t[:, :],
                                             start=False, stop=True)
                        if "A" in phases:
                            eg = SM.tile([LQ, H], F32, tag="eg")
                            nc.scalar.activation(eg[:, :], psg[:, :], AF.Exp, scale=-1.0)
                            eg1 = SM.tile([LQ, H], F32, tag="eg1")
                            nc.vector.tensor_scalar(eg1[:, :], eg[:, :], 1.0, None, ALU.add)
                            nc.vector.reciprocal(gate[:, :], eg1[:, :])
                        # ones column of v_sb
                        ones_ap = v_sb[:, :].rearrange("p (kt h x) -> p kt h x", kt=KT, h=H)[:, :, :, 32:33]
                        nc.vector.memset(ones_ap, 1.0)
                        if DEBUG:
                            nc.sync.dma_start(out=d_sT[:, :], in_=sT[:, :])

                # ---------------- Phase B: pair -> biasT ---------------------
                if "B" not in phases:
                    nc.vector.memset(biasT[:, :], 0.0)
                A2_AT = 9   # emit A2's matmuls after this many B tiles
                import contextlib as _ctxlib
                # Phase C's pools are OPENED first (bank reservation is lazy,
                # at first tile alloc) so the B pools sit on top of the pool
                # stack and can be released mid-emission, LIFO-correctly,
                # before the final-projection pool opens.
                es_C = _ctxlib.ExitStack()
                CSM = es_C.enter_context(tc.tile_pool(name="pcsm", bufs=4))
                PR = es_C.enter_context(tc.tile_pool(name="probs", bufs=8))
                es_B = _ctxlib.ExitStack()
                SQ = es_B.enter_context(tc.tile_pool(name="sqp", bufs=3))
                SM = es_B.enter_context(tc.tile_pool(name="pbsm", bufs=12))
                PSB = es_B.enter_context(
                    tc.tile_pool(name="psB", bufs=3, space="PSUM"))
                PSS = es_B.enter_context(
                    tc.tile_pool(name="psS", bufs=2, space="PSUM"))
                if True:
                    # DVE squares [0:SPLIT], ACT squares the rest — into TWO
                    # separate tiles: with subtile deps disabled, two engines
                    # writing halves of one tile get WAW-serialized and the
                    # whole per-tile chain stops pipelining.
                    SQ_SPLIT = 1280

                    def emit_b_tile(lb):
                        pt2 = pt_tiles[lb]
                        sqD = SQ.tile([128, SQ_SPLIT], BF16, tag="sqD")
                        sqA = SQ.tile([128, 2 * L - SQ_SPLIT], BF16, tag="sqA")
                        nc.vector.tensor_tensor(sqD[:, :], pt2[:, 0:SQ_SPLIT],
                                                pt2[:, 0:SQ_SPLIT], ALU.mult)
                        nc.scalar.activation(sqA[:, :], pt2[:, SQ_SPLIT:],
                                             AF.Square)
                        # N=17 matmuls: 16 bias cols + mean (wbc col16=1/128)
                        # into ONE psum bank [p, (ls2, kt, 17)]; E[x^2] via
                        # N=1 matmuls on the squared tiles into a stats bank.
                        # All stats then batch per-tile: 1 square, 1 subtract,
                        # 1 ln, 1 exp, 1 scale.
                        bB = PSB.tile([128, 2 * KT * 17], F32, tag="bB")
                        st = PSS.tile([128, 2 * KT], F32, tag="st")   # E2
                        for ls2 in range(2):
                            pT = pt2[:, ls2 * L:(ls2 + 1) * L]
                            for kt in range(KT):
                                nc.tensor.matmul(
                                    bB[:, (ls2 * KT + kt) * 17:
                                       (ls2 * KT + kt + 1) * 17],
                                    pT[:, kt * 128:(kt + 1) * 128],
                                    wbc_t[:, :], start=True, stop=True)
                            for kt in range(KT):
                                col = ls2 * L + kt * 128
                                sqsrc = (sqD[:, col:col + 128] if col < SQ_SPLIT
                                         else sqA[:, col - SQ_SPLIT:
                                                  col - SQ_SPLIT + 128])
                                nc.tensor.matmul(
                                    st[:, ls2 * KT + kt: ls2 * KT + kt + 1],
                                    sqsrc, wbc_t[:, 16:17], start=True, stop=True)
                        # m^2, layout [p, (ls2, kt)] matching st
                        m2 = SM.tile([128, 2 * KT], F32, tag="m2")
                        nc.scalar.activation(
                            m2[:, :].rearrange("p (ls2 kt) -> p ls2 kt", ls2=2),
                            bB[:, :].rearrange("p (ls2 kt c) -> p ls2 kt c",
                                               ls2=2, kt=KT)[:, :, :, 16],
                            AF.Square)
                        ve4 = SM.tile([128, 2 * KT], F32, tag="ve4")
                        nc.vector.tensor_tensor(ve4[:, :], st[:, :], m2[:, :],
                                                ALU.subtract)
                        # batched rsqrt via exp(-0.5*ln(x + eps))
                        lv4 = SM.tile([128, 2 * KT], F32, tag="lv4")
                        nc.scalar.activation(lv4[:, :], ve4[:, :], AF.Ln,
                                             bias=eps_c[:, :])
                        si4 = SM.tile([128, 2 * KT], F32, tag="si4")
                        nc.scalar.activation(si4[:, :], lv4[:, :], AF.Exp,
                                             scale=-0.5)
                        # scale: biasT[p, kt, lb*2+ls2, h] = bB[p, ls2, kt, 0:16]*si
                        y_ap = bB[:, :].rearrange("p (ls2 kt c) -> p kt ls2 c",
                                                  ls2=2, kt=KT)[:, :, :, 0:16]
                        s_ap = si4[:, :].rearrange("p (ls2 kt o) -> p kt ls2 o",
                                                   ls2=2, o=1).to_broadcast(
                                                       (128, KT, 2, H))
                        b_ap = biasT[:, :].rearrange(
                            "p (kt l h) -> p kt l h", kt=KT,
                            l=LQ)[:, :, lb * 2:(lb + 1) * 2, :]
                        nc.vector.tensor_tensor(b_ap, y_ap, s_ap, ALU.mult)

                    nb_tiles = LQ // 2 if "B" in phases else 0
                    for lb in range(min(A2_AT, nb_tiles)):
                        emit_b_tile(lb)
                    emit_a2()
                    for lb in range(min(A2_AT, nb_tiles), min(nb_tiles, 32)):
                        emit_b_tile(lb)

                # ---------------- Phase C: attention ------------------------
                # Emitted per l-HALF (64 query rows). Half 0's bias rows are
                # complete once B tile 31 lands, so its 16 heads interleave
                # with B tiles 32..63 — phase C's PE work runs under phase B's
                # DMA-paced stretch instead of serially after it. PSUM during
                # the interleave: B(3+2) + lg(2) + oLV(1) = 8 banks exactly;
                # the B pools close before the final-projection pool opens.
                LH = LQ // 2
                if True:
                    id_b = None
                    if "C" in phases:
                        id_b = CSM.tile([128, 128], BF16, tag="idb")
                        nc.vector.tensor_copy(id_b[:, :], id_t[:, :])

                    def emit_c_head(h, lh, PSL, PSO):
                        mc, i0 = h // 4, (h % 4) * 32
                        oLV = PSO.tile([LQ, 33], F32, tag="oLV")
                        oLVs = oLV[lh * LH:(lh + 1) * LH, :]
                        for kg in range(2):      # two [128, 4*LH] logit banks
                            lg = PSL.tile([128, 4 * LH], F32, tag="lg")
                            # Seed the bank with the bias in ONE identity
                            # matmul (start=True sets has_written — a DVE
                            # preload would be OVERWRITTEN by the first
                            # accumulating matmul on TRN2); kq accumulates.
                            b_ap = biasT[:, kg * 4 * (LQ * H):
                                         (kg + 1) * 4 * (LQ * H)].rearrange(
                                "p (kt l h) -> p kt l h", kt=4,
                                l=LQ)[:, :, lh * LH:(lh + 1) * LH, h]
                            nc.tensor.matmul(
                                lg[:, :].rearrange("p (kt l) -> p kt l", kt=4),
                                id_b[:, :], b_ap,
                                start=True, stop=False, skip_group_check=True)
                            for sub in range(4):
                                kt = kg * 4 + sub
                                sl = lg[:, sub * LH:(sub + 1) * LH]
                                nc.tensor.matmul(
                                    sl,
                                    kTb[i0:i0 + 32, mc * L + kt * 128: mc * L + (kt + 1) * 128],
                                    qTb[i0:i0 + 32, mc * LQ + lh * LH: mc * LQ + (lh + 1) * LH],
                                    start=False, stop=True, tile_position=(i0, 0),
                                    skip_group_check=True)
                            pr = PR.tile([128, 4 * LH], BF16, tag="pr")
                            if use_mask:
                                for sub in range(4):
                                    kt = kg * 4 + sub
                                    nc.scalar.activation(
                                        pr[:, sub * LH:(sub + 1) * LH],
                                        lg[:, sub * LH:(sub + 1) * LH], AF.Exp,
                                        bias=maskb_t[:, kt:kt + 1])
                            else:
                                nc.scalar.activation(pr[:, :], lg[:, :], AF.Exp)
                            for sub in range(4):
                                kt = kg * 4 + sub
                                nc.tensor.matmul(
                                    oLVs, pr[:, sub * LH:(sub + 1) * LH],
                                    v_sb[:, kt * (H * 33) + h * 33: kt * (H * 33) + (h + 1) * 33],
                                    start=(kt == 0), stop=(kt == KT - 1),
                                    skip_group_check=True)
                        dv_c = CSM.tile([LQ, 1], F32, tag="dv")
                        dv = dv_c[lh * LH:(lh + 1) * LH, :]
                        nc.vector.reciprocal(dv, oLVs[:, 32:33])
                        gd_c = CSM.tile([LQ, 1], F32, tag="gd")
                        gd = gd_c[lh * LH:(lh + 1) * LH, :]
                        nc.vector.tensor_tensor(
                            gd, gate[lh * LH:(lh + 1) * LH, h:h + 1], dv, ALU.mult)
                        nc.vector.tensor_scalar(
                            outN_g[h // 4][lh * LH:(lh + 1) * LH,
                                           (h % 4) * DH:(h % 4 + 1) * DH],
                            oLVs[:, 0:32], gd, None, ALU.mult)

                    interleave = "C" in phases and nb_tiles == LQ // 2
                    if interleave:
                        # half-0's pools: B(3+2) + lg(2) + oLV(1) = 8 banks
                        with (
                            tc.tile_pool(name="psL0", bufs=2, space="PSUM") as PSL0,
                            tc.tile_pool(name="psO0", bufs=1, space="PSUM") as PSO0,
                        ):
                            for k in range(H):
                                emit_c_head(k, 0, PSL0, PSO0)
                                emit_b_tile(32 + 2 * k)
                                emit_b_tile(33 + 2 * k)
                    else:
                        for lb in range(min(nb_tiles, 32), nb_tiles):
                            emit_b_tile(lb)
                    es_B.close()   # release phase B's 5 PSUM banks

                    with (
                        tc.tile_pool(name="psL1", bufs=4, space="PSUM") as PSL1,
                        tc.tile_pool(name="psO1", bufs=2, space="PSUM") as PSO1,
                        tc.tile_pool(name="psF", bufs=1, space="PSUM") as PSF,
                    ):
                        po = PSF.tile([LQ, D], F32, tag="po")

                        def emit_final_group(g):
                            # transpose this head-group's gated output and fold
                            # it into the Wo accumulation while later heads run
                            psT = PSF.tile([128, 4 * DH], F32, tag="psT")
                            nc.tensor.transpose(psT[:, :], outN_g[g][:, :],
                                                id_t[:, :])
                            nc.vector.tensor_copy(outg_g[g][:, :], psT[:, :])
                            nc.tensor.matmul(
                                po[:, :], outg_g[g][:, :],
                                wo_t[:, g * D:(g + 1) * D],
                                start=(g == 0), stop=(g == DC - 1),
                                skip_group_check=True)

                        if "C" in phases:
                            if not interleave:
                                for h in range(H):
                                    emit_c_head(h, 0, PSL1, PSO1)
                            for h in range(H):
                                emit_c_head(h, 1, PSL1, PSO1)
                                if h % 4 == 3:
                                    emit_final_group(h // 4)
                        else:
                            for g in range(DC):
                                nc.vector.memset(outN_g[g][:, :], 0.0)
                                emit_final_group(g)
                        nc.vector.tensor_copy(out_f[:, :], po[:, :])
                        # column-split the result DMA over 4 queues (one 256KB
                        # transfer would sit ~11us on a single queue at the
                        # very end of the kernel)
                        for j4 in range(DC):
                            nc.sync.dma_start(
                                out=out[:, j4 * 128:(j4 + 1) * 128],
                                in_=out_f[:, j4 * 128:(j4 + 1) * 128])
                    if DEBUG:
                        nc.sync.dma_start(out=d_gate[:, :], in_=gate[:, :])
                        nc.sync.dma_start(out=d_kTb[:, :], in_=kTb[:, :])
                        nc.sync.dma_start(out=d_qTb[:, :], in_=qTb[:, :])
                        nc.sync.dma_start(out=d_biasT[:, :], in_=biasT[:, :])
                        for g in range(DC):
                            nc.sync.dma_start(out=d_outN[:, g * 128:(g + 1) * 128],
                                              in_=outN_g[g][:, :])
                        nc.sync.dma_start(out=d_vsb[:, :], in_=v_sb[:, :])
                    es_C.close()
    nc.compile()
    return nc


def _prep_inputs(single, pair, mask, ln_s_g, ln_s_b, Wq, bq, Wk, Wv,
                 ln_p_g, ln_p_b, Wb, Wg, Wo):
    f32 = np.float32
    single = np.asarray(single, f32).reshape(L, D)
    pair = np.asarray(pair, f32).reshape(L, L, P)
    maskv = np.asarray(mask).reshape(L).astype(bool)
    g_s = np.asarray(ln_s_g, f32); b_s = np.asarray(ln_s_b, f32)
    g_p = np.asarray(ln_p_g, f32)
    Wq = np.asarray(Wq, f32); Wk = np.asarray(Wk, f32); Wv = np.asarray(Wv, f32)
    Wg = np.asarray(Wg, f32); Wo = np.asarray(Wo, f32); Wb = np.asarray(Wb, f32)
    bq = np.asarray(bq, f32)

    sc = DH ** -0.5
    Wq2 = (g_s[:, None] * Wq) * sc
    bq2 = (b_s @ Wq + bq) * sc
    Wk2 = g_s[:, None] * Wk; bk2 = b_s @ Wk
    Wv2 = g_s[:, None] * Wv; bv2 = b_s @ Wv
    Wg2 = g_s[:, None] * Wg; bg2 = b_s @ Wg
    Wb2 = g_p[:, None] * Wb
    Wbc = Wb2 - Wb2.mean(0, keepdims=True)          # [128, 16]
    wbc_host = np.concatenate([Wbc, np.full((P, 1), 1.0 / P, f32)], axis=1)

    def pack_lhsT(W):   # [512, M] -> [128, 4*M] with (dc, mc-major cols)
        Din, M = W.shape
        return W.reshape(4, 128, M).transpose(1, 0, 2).reshape(128, 4 * M)

    bf = ml_dtypes.bfloat16
    wq_h = pack_lhsT(Wq2).astype(bf); wk_h = pack_lhsT(Wk2).astype(bf)
    wv_h = pack_lhsT(Wv2).astype(bf)
    wg_h = pack_lhsT(Wg2).astype(bf); wo_h = pack_lhsT(Wo).astype(bf)
    bq_h = bq2.reshape(4, 128).T.copy()
    bk_h = bk2.reshape(4, 128).T.copy()
    bv_h = np.broadcast_to(bv2, (128, D)).astype(bf)
    bgn_h = bg2.reshape(1, H).astype(bf)

    maskbias = np.where(maskv, 0.0, -1e9).astype(f32)
    pair_bf = pair.astype(ml_dtypes.bfloat16)

    in_maps = []
    for cid in range(NC):
        sh = -cid * LQ
        # Pre-transpose the core's pair slice to [lb][p][(ls2, k)] so the
        # device DMA is fully linear (4KB per partition row).
        sl = np.roll(pair_bf[cid * LQ:(cid + 1) * LQ], sh, axis=1)
        ptc = sl.transpose(2, 0, 1).reshape(128, LQ // 2, 2 * L)
        ptc = np.ascontiguousarray(ptc.transpose(1, 0, 2))
        in_maps.append({
            "pair_t": ptc,
            "single": np.roll(single, sh, axis=0).astype(ml_dtypes.bfloat16),
            "wq": wq_h, "wk": wk_h, "wv": wv_h, "wg": wg_h, "wo": wo_h,
            "wbc": wbc_host.astype(ml_dtypes.bfloat16),
            "bq": bq_h, "bk": bk_h, "bv": bv_h, "bgn": bgn_h,
            "maskb": np.roll(maskbias, sh).reshape(KT, 128).T.copy(),
            "ident": np.eye(128, dtype=f32),
            "out": np.zeros((LQ, D), f32),
            **({"d_gate": np.zeros((LQ, H), f32),
                "d_kTb": np.zeros((128, 4 * L), ml_dtypes.bfloat16),
                "d_qTb": np.zeros((128, 4 * LQ), ml_dtypes.bfloat16),
                "d_biasT": np.zeros((128, KT * LQ * H), ml_dtypes.bfloat16),
                "d_outN": np.zeros((LQ, D), f32),
                "d_vsb": np.zeros((128, KT * H * 33), ml_dtypes.bfloat16),
                "d_sT": np.zeros((128, 4 * L), ml_dtypes.bfloat16)} if DEBUG else {}),
        })
    return in_maps


def kernel(**inputs):
    use_mask = not np.asarray(inputs["mask"]).reshape(-1).astype(bool).all()
    key = ("nc", use_mask)
    if key not in _CACHED:
        _CACHED[key] = _build_bass(use_mask=use_mask)
    nc = _CACHED[key]
    in_maps = _prep_inputs(**inputs)
    res = run_bass_kernel_spmd(nc, in_maps, list(range(NC)),
                               trace=bool(LAST_INFO.get("want_trace")))
    LAST_INFO["results"] = res
    outs = [np.asarray(res.results[i]["out"]) for i in range(NC)]
    return np.concatenate(outs, axis=0).reshape(B, L, D).astype(np.float32)



# revision 61
# speedup vs baseline: 1.0133x; 1.0133x over previous
"""AttentionPairBias Trainium2 kernel (8 NeuronCores, query-sharded).

Strategy:
  - Shard the 1024 query rows across 8 cores (128 rows each). Each core reads
    only its slice of the huge pair tensor (512MB/8 = 64MB f32 -> 32MB bf16).
  - Host folds both LayerNorm affine transforms into the projection weights,
    centers the pair->bias weights so the pair-LN mean correction is free, and
    converts the pair slice to bf16 (DMA halved, enables DMA-transpose loads).
  - Per-head bias constant (ln_p_b @ Wb) is dropped: constant per (l,h) row is
    softmax-invariant.
  - On device, pair tiles arrive TRANSPOSED ([p, k] layout) via the DMA xbar,
    so the bias matmul contracts p on the PE with the pair tile as the
    stationary operand, producing [k, h] tiles; LN stats (mean / E[x^2]) come
    from extra matmul columns against ones/128; variance -> rsqrt is done as
    exp(-0.5*ln(var+eps)) so the whole kernel uses one ACT table set.
  - Attention runs transposed: logits^T[k,l] per (head, ktile), probs = exp()
    with the key-mask folded into the ACT bias operand, attn@v uses probs as
    the moving operand with a fused ones-column producing the softmax
    denominator for free. Output is built transposed, feeding the final Wo
    matmul without any extra transpose.
"""

import os

os.environ.setdefault("MYCRO_LOCAL_CACHE", "1")
# Tile's subtile dependency tracker mishandles interleaved strided APs (e.g.
# the [p, (dc, l)] transposed-activation writes) and lets consumers run before
# all producers; whole-tile deps are correct and cost nothing here since the
# kernel's phases are naturally sequential.
os.environ["BY_DEFAULT_DISABLE_SUBTILE_DEPS"] = "1"

import numpy as np
import ml_dtypes

# bass_utils imports antenv.axon_hooks unguarded when tracing is requested
# (e.g. BASS_TRACE=1 in the environment); some images lack that submodule.
# Provide the graceful no-hook fallback instead of an ImportError.
try:
    import antenv.axon_hooks  # noqa: F401
except ImportError:
    import sys as _sys
    import types as _types

    try:
        import antenv as _antenv
        _m = _types.ModuleType("antenv.axon_hooks")
        _hook = [None]
        _m.set_axon_ntff_profile_hook = lambda h: _hook.__setitem__(0, h)
        _m.get_axon_ntff_profile_hook = lambda: _hook[0]
        _sys.modules["antenv.axon_hooks"] = _m
        _antenv.axon_hooks = _m
    except ImportError:
        pass

# Prefer the ACT table set that contains Exp, Ln AND Square so the whole
# kernel needs exactly one table load. With the default set ordering the
# chooser alternates between an Exp/Square set and an Ln set inside the main
# loop, inserting ~270 table loads (~2.7us each).
import concourse.hw_specs as _hw_specs

_orig_get_act_tables = _hw_specs.get_activation_tables

def _patched_get_act_tables(arch):
    # Keep dict ORDER intact (set ids are positional — walrus loads tables by
    # index), but make natural_log_exp_and_others the only set offering Exp,
    # Ln and Square so every activation in this kernel resolves to one set.
    tabs = _orig_get_act_tables(arch)
    pref = "natural_log_exp_and_others"
    if pref not in tabs:
        return tabs
    strip = tabs[pref]
    return {
        k: (v if k == pref else (v - strip)) for k, v in tabs.items()
    }

_hw_specs.get_activation_tables = _patched_get_act_tables

import concourse.bass as bass
import concourse.bacc as bacc
import concourse.mybir as mybir
from concourse.bass_utils import run_bass_kernel_spmd
from concourse.tile import TileContext

F32 = mybir.dt.float32
F32R = mybir.dt.float32r
BF16 = mybir.dt.bfloat16
AF = mybir.ActivationFunctionType
ALU = mybir.AluOpType
AX = mybir.AxisListType

B, L, D, P, H = 1, 1024, 512, 128, 16
DH = D // H          # 32
NC = 8               # cores
LQ = L // NC         # 128 query rows per core
KT = L // 128        # 8 key tiles
DC = D // 128        # 4 D chunks
EPS = 1e-5

_CACHED = {}
LAST_INFO = {}
DEBUG = False


def _build_bass(phases="ABC", loop_n=None, use_mask=False):
    nc = bacc.Bacc("TRN2", target_bir_lowering=False, debug=False)
    # pair arrives HOST-pre-transposed: tile lb holds [p, (ls, k)] for the 4
    # query rows lb*4..lb*4+3 — a fully linear 1MB DMA (8KB per partition
    # row). The on-device DMA-transpose path ran at ~220GB/s and paced the
    # whole front half of the kernel; linear loads run at full HBM rate.
    pair_t = nc.declare_dram_parameter("pair_t", [LQ // 2, 128, 2 * L], BF16,
                                       isOutput=False)
    single = nc.declare_dram_parameter("single", [L, D], BF16, isOutput=False)
    wq = nc.declare_dram_parameter("wq", [128, 4 * D], BF16, isOutput=False)
    wk = nc.declare_dram_parameter("wk", [128, 4 * D], BF16, isOutput=False)
    wv = nc.declare_dram_parameter("wv", [128, 4 * D], BF16, isOutput=False)
    wg = nc.declare_dram_parameter("wg", [128, 4 * H], BF16, isOutput=False)
    wo = nc.declare_dram_parameter("wo", [128, 4 * D], BF16, isOutput=False)
    wbc = nc.declare_dram_parameter("wbc", [128, 17], BF16, isOutput=False)
    bq = nc.declare_dram_parameter("bq", [128, 4], F32, isOutput=False)
    bk = nc.declare_dram_parameter("bk", [128, 4], F32, isOutput=False)
    bv = nc.declare_dram_parameter("bv", [128, D], BF16, isOutput=False)
    bgn = nc.declare_dram_parameter("bgn", [1, H], BF16, isOutput=False)
    maskb = nc.declare_dram_parameter("maskb", [128, KT], F32, isOutput=False)
    ident = nc.declare_dram_parameter("ident", [128, 128], F32, isOutput=False)
    out = nc.declare_dram_parameter("out", [LQ, D], F32, isOutput=True)
    if DEBUG:
        d_gate = nc.declare_dram_parameter("d_gate", [LQ, H], F32, isOutput=True)
        d_kTb = nc.declare_dram_parameter("d_kTb", [128, 4 * L], BF16, isOutput=True)
        d_qTb = nc.declare_dram_parameter("d_qTb", [128, 4 * LQ], BF16, isOutput=True)
        d_biasT = nc.declare_dram_parameter("d_biasT", [128, KT * LQ * H], BF16, isOutput=True)
        d_outN = nc.declare_dram_parameter("d_outN", [LQ, D], F32, isOutput=True)
        d_vsb = nc.declare_dram_parameter("d_vsb", [128, KT * H * 33], BF16, isOutput=True)
        d_sT = nc.declare_dram_parameter("d_sT", [128, 4 * L], BF16, isOutput=True)

    with TileContext(nc) as tc:
        with tc.tile_pool(name="persist", bufs=1) as PS:
            kTb = PS.tile([128, 4 * L], BF16)        # [dk%128, (mc, k)]
            qTb = PS.tile([128, 4 * LQ], BF16)       # [dq%128, (mc, l)]
            v_sb = PS.tile([128, KT * (H * 33)], BF16)  # per kt: 16h x (32 v | 1 one)
            # bias, split per l-HALF so phase C's half-0 readers touch a
            # different tile than phase B's second-half writers (whole-tile
            # WAR edges would otherwise serialize the interleave)
            biasT_g = [PS.tile([128, KT * (LQ // 2) * H], BF16,
                               name=f"biasT{i}") for i in range(2)]
            gate = PS.tile([LQ, H], F32)
            wbc_t = PS.tile([128, 17], BF16)
            maskb_t = PS.tile([128, KT], F32)
            # weights split per-dc chunk: consumers read per-dc slices anyway,
            # and 4 separate 128KB DMAs spread across 4 queues instead of one
            # 512KB transfer camping on a single queue (~23us)
            wo_t = PS.tile([128, 4 * D], BF16)
            # gated attn out, split per 4-head group so the final transpose +
            # Wo accumulation can start as soon as its group's heads finish
            # (whole-tile deps would otherwise stall them to the very end)
            outN_g = [PS.tile([LQ, 4 * DH], F32, name=f"outN{dc}") for dc in range(DC)]
            outg_g = [PS.tile([128, LQ], BF16, name=f"outg{dc}") for dc in range(DC)]
            out_f = PS.tile([LQ, D], F32)
            id_t = PS.tile([128, 128], F32)
            eps_c = PS.tile([128, 1], F32)

            # Only the loads phase A1/B1 need immediately are issued here;
            # everything else is deferred behind the head-critical x + pair
            # triggers (each dma_start costs ~650ns of serial Sync-queue time,
            # so trigger ORDER sets the pipeline ramp).
            nc.sync.dma_start(out=id_t[:, :], in_=ident[:, :])
            nc.sync.dma_start(out=wbc_t[:, :], in_=wbc[:, :])
            nc.vector.memset(eps_c[:, :], EPS)
            import contextlib
            _loop_cm = tc.For_i(0, loop_n, 1) if loop_n else contextlib.nullcontext()
            with (
                _loop_cm,
                tc.tile_pool(name="pairp", bufs=16) as PP,
                tc.tile_pool(name="paw", bufs=1) as WW,
                tc.tile_pool(name="pax", bufs=8) as PX,
            ):
                # The first few pair tiles are issued BEFORE everything else:
                # their ~22us single-queue latency gates phase B's start, while
                # phase A tolerates its inputs arriving a few us later.
                npair = LQ // 2 if ("B" in phases or "D" in phases) else 0
                pt_tiles = []

                def emit_pair_dma(lb):
                    pt2 = PP.tile([128, 2 * L], BF16, tag="pt2")
                    nc.sync.dma_start(out=pt2[:, :], in_=pair_t[lb])
                    pt_tiles.append(pt2)

                # x tiles first (phase A starts off x[0]), then the pair head
                x_tiles = []
                for lt in range(L // 128 if "A" in phases else 0):
                    x = PX.tile([128, D], BF16, tag="x")
                    nc.sync.dma_start(out=x[:, :], in_=single[lt * 128:(lt + 1) * 128, :])
                    x_tiles.append(x)
                NPRE = 6
                for lb in range(min(NPRE, npair)):
                    emit_pair_dma(lb)
                # weights: not needed until A2 / phase C
                wq_t = WW.tile([128, 4 * D], BF16)
                wk_t = WW.tile([128, 4 * D], BF16)
                wv_t = WW.tile([128, 4 * D], BF16)
                wg_t = WW.tile([128, 4 * H], BF16)
                bq_t = WW.tile([128, 4], F32)
                bk_t = WW.tile([128, 4], F32)
                bv_t = WW.tile([128, D], BF16)
                bg_t = WW.tile([1, H], BF16)
                ones_t = WW.tile([1, LQ], BF16)
                nc.sync.dma_start(out=wq_t[:, :], in_=wq[:, :])
                nc.sync.dma_start(out=wk_t[:, :], in_=wk[:, :])
                nc.sync.dma_start(out=wv_t[:, :], in_=wv[:, :])
                nc.sync.dma_start(out=wg_t[:, :], in_=wg[:, :])
                nc.sync.dma_start(out=bq_t[:, :], in_=bq[:, :])
                nc.sync.dma_start(out=bk_t[:, :], in_=bk[:, :])
                nc.sync.dma_start(out=bv_t[:, :], in_=bv[:, :])
                nc.sync.dma_start(out=bg_t[:, :], in_=bgn[:, :])
                nc.sync.dma_start(out=maskb_t[:, :], in_=maskb[:, :])
                nc.sync.dma_start(out=wo_t[:, :], in_=wo[:, :])
                nc.vector.memset(ones_t[:, :], 1.0)
                # Remaining pair loads (pre-transposed on host). Each dma_start
                # lands on ONE of the 16 queues (~22GB/s each), so tile size
                # sets the latency-to-first-tile: half-size 512KB tiles (2
                # query rows) arrive in ~22us, and a 16-deep pool covers the
                # bandwidth-delay product so the stream never starves.
                # (Partition-split sub-DMAs are NOT used: <128-partition
                # transfers lose AXI ports to the swizzle and run ~1.6x
                # slower. Compute-queue issue is also out: a pool-gated DMA
                # trigger would stall all compute behind it.)
                for lb in range(NPRE, npair):
                    emit_pair_dma(lb)

                if "D" in phases and "B" not in phases:
                    with tc.tile_pool(name="dsink", bufs=2) as DS:
                        for lb in range(LQ // 2):
                            snk = DS.tile([128, 1], BF16, tag="snk")
                            nc.vector.tensor_copy(snk[:, :], pt_tiles[lb][:, 0:1])
                # ---------------- Phase A1: LN(single) -> sT -----------------
                # Engine queues are in-order, so phase A's PE instructions
                # ahead of phase B's would delay B ~30us past its data. Emit
                # A1 (cheap transposes), then the first B tiles, then A2 (the
                # projection matmuls) under B's engine slack, then the rest
                # of B.
                sT = PS.tile([128, 4 * L], BF16)  # xhat^T: [D%128, (dc, l)]
                with (
                    tc.tile_pool(name="pa", bufs=2) as PA,
                    tc.tile_pool(name="pasm", bufs=3) as SM,
                    tc.tile_pool(name="paps", bufs=2, space="PSUM") as PSA,
                ):
                    for lt in range(L // 128 if "A" in phases else 0):
                        x = x_tiles[lt]          # bf16 (upconverted on read)
                        S = SM.tile([128, 1], F32, tag="S")
                        nc.vector.tensor_reduce(S[:, :], x[:, :], AX.X, ALU.add)
                        sq = PA.tile([128, D], F32, tag="sq")
                        Q = SM.tile([128, 1], F32, tag="Q")
                        nc.scalar.activation(sq[:, :], x[:, :], AF.Square, accum_out=Q[:, :])
                        m = SM.tile([128, 1], F32, tag="m")
                        nc.vector.tensor_scalar(m[:, :], S[:, :], 1.0 / D, None, ALU.mult)
                        m2 = SM.tile([128, 1], F32, tag="m2")
                        nc.vector.tensor_tensor(m2[:, :], m[:, :], m[:, :], ALU.mult)
                        q2 = SM.tile([128, 1], F32, tag="q2")
                        nc.vector.tensor_scalar(q2[:, :], Q[:, :], 1.0 / D, EPS, ALU.mult, ALU.add)
                        ve = SM.tile([128, 1], F32, tag="ve")
                        nc.vector.tensor_tensor(ve[:, :], q2[:, :], m2[:, :], ALU.subtract)
                        lv = SM.tile([128, 1], F32, tag="lv")
                        nc.scalar.activation(lv[:, :], ve[:, :], AF.Ln)
                        sinv = SM.tile([128, 1], F32, tag="si")
                        nc.scalar.activation(sinv[:, :], lv[:, :], AF.Exp, scale=-0.5)
                        xh = PA.tile([128, D], F32, tag="xh")
                        nc.vector.tensor_scalar(xh[:, :], x[:, :], m[:, :], sinv[:, :],
                                                ALU.subtract, ALU.mult)
                        pst = PSA.tile([128, D], F32, tag="pst")
                        for j in range(DC):
                            nc.tensor.transpose(pst[:, j * 128:(j + 1) * 128],
                                                xh[:, j * 128:(j + 1) * 128], id_t[:, :])
                        o_ap = sT[:, :].rearrange("p (dc n) -> p dc n", dc=DC)[:, :, lt * 128:(lt + 1) * 128]
                        i_ap = pst[:, :].rearrange("p (dc j) -> p dc j", dc=DC)
                        nc.vector.tensor_copy(o_ap, i_ap)

                def emit_a2():
                    # ------------ Phase A2: projections off sT ---------------
                    with (
                        tc.tile_pool(name="pa2sm", bufs=3) as SM,
                        tc.tile_pool(name="paps2", bufs=1, space="PSUM") as PSA,
                    ):
                        # kT (keys, transposed, bf16)
                        for mc in range(4 if "A" in phases else 0):
                            for nb in range(2):
                                ps = PSA.tile([128, 512], F32, tag="kv")
                                for dc in range(DC):
                                    nc.tensor.matmul(
                                        ps[:, :],
                                        wk_t[:, dc * D + mc * 128: dc * D + (mc + 1) * 128],
                                        sT[:, dc * L + nb * 512: dc * L + (nb + 1) * 512],
                                        start=(dc == 0), stop=(dc == DC - 1))
                                nc.vector.tensor_scalar(
                                    kTb[:, mc * L + nb * 512: mc * L + (nb + 1) * 512],
                                    ps[:, :], bk_t[:, mc:mc + 1], None, ALU.add)
                        # v (natural layout, h-interleaved with ones column)
                        for kt in range(KT if "A" in phases else 0):
                            ps = PSA.tile([128, 512], F32, tag="kv")
                            for dc in range(DC):
                                nc.tensor.matmul(
                                    ps[:, :],
                                    sT[:, dc * L + kt * 128: dc * L + (kt + 1) * 128],
                                    wv_t[:, dc * D:(dc + 1) * D],
                                    start=(dc == 0), stop=(dc == DC - 1))
                            o_ap = v_sb[:, kt * (H * 33):(kt + 1) * (H * 33)].rearrange(
                                "p (h x) -> p h x", h=H)[:, :, 0:32]
                            nc.vector.tensor_tensor(
                                o_ap, ps[:, :].rearrange("p (h x) -> p h x", h=H),
                                bv_t[:, :].rearrange("p (h x) -> p h x", h=H), ALU.add)
                        # qT for own 128 rows (device rows 0..127 via host roll)
                        for mc in range(4 if "A" in phases else 0):
                            ps = PSA.tile([128, LQ], F32, tag="q")
                            for dc in range(DC):
                                nc.tensor.matmul(
                                    ps[:, :],
                                    wq_t[:, dc * D + mc * 128: dc * D + (mc + 1) * 128],
                                    sT[:, dc * L: dc * L + LQ],
                                    start=(dc == 0), stop=(dc == DC - 1))
                            nc.vector.tensor_scalar(
                                qTb[:, mc * LQ:(mc + 1) * LQ], ps[:, :],
                                bq_t[:, mc:mc + 1], None, ALU.add)
                        # gate = sigmoid(xhat @ Wg + bg) = 1/(1+exp(-x-bg))
                        if "A" not in phases:
                            nc.vector.memset(gate[:, :], 0.5)
                        psg = PSA.tile([LQ, H], F32, tag="g", name="psg") if "A" in phases else None
                        for dc in range(DC if "A" in phases else 0):
                            nc.tensor.matmul(
                                psg[:, :],
                                sT[:, dc * L: dc * L + LQ],
                                wg_t[:, dc * H:(dc + 1) * H],
                                start=(dc == 0), stop=False)
                        if "A" in phases:
                            nc.tensor.matmul(psg[:, :], ones_t[:, :], bg_t[:, :],
                                             start=False, stop=True)
                        if "A" in phases:
                            eg = SM.tile([LQ, H], F32, tag="eg")
                            nc.scalar.activation(eg[:, :], psg[:, :], AF.Exp, scale=-1.0)
                            eg1 = SM.tile([LQ, H], F32, tag="eg1")
                            nc.vector.tensor_scalar(eg1[:, :], eg[:, :], 1.0, None, ALU.add)
                            nc.vector.reciprocal(gate[:, :], eg1[:, :])
                        # ones column of v_sb
                        ones_ap = v_sb[:, :].rearrange("p (kt h x) -> p kt h x", kt=KT, h=H)[:, :, :, 32:33]
                        nc.vector.memset(ones_ap, 1.0)
                        if DEBUG:
                            nc.sync.dma_start(out=d_sT[:, :], in_=sT[:, :])

                # ---------------- Phase B: pair -> biasT ---------------------
                if "B" not in phases:
                    nc.vector.memset(biasT_g[0][:, :], 0.0)
                    nc.vector.memset(biasT_g[1][:, :], 0.0)
                A2_AT = 9   # emit A2's matmuls after this many B tiles
                import contextlib as _ctxlib
                # Phase C's pools are OPENED first (bank reservation is lazy,
                # at first tile alloc) so the B pools sit on top of the pool
                # stack and can be released mid-emission, LIFO-correctly,
                # before the final-projection pool opens.
                es_C = _ctxlib.ExitStack()
                CSM = es_C.enter_context(tc.tile_pool(name="pcsm", bufs=4))
                PR = es_C.enter_context(tc.tile_pool(name="probs", bufs=8))
                es_B = _ctxlib.ExitStack()
                SQ = es_B.enter_context(tc.tile_pool(name="sqp", bufs=3))
                SM = es_B.enter_context(tc.tile_pool(name="pbsm", bufs=12))
                PSB = es_B.enter_context(
                    tc.tile_pool(name="psB", bufs=3, space="PSUM"))
                PSS = es_B.enter_context(
                    tc.tile_pool(name="psS", bufs=2, space="PSUM"))
                if True:
                    # DVE squares [0:SPLIT], ACT squares the rest — into TWO
                    # separate tiles: with subtile deps disabled, two engines
                    # writing halves of one tile get WAW-serialized and the
                    # whole per-tile chain stops pipelining.
                    SQ_SPLIT = 1280

                    def emit_b_tile(lb):
                        pt2 = pt_tiles[lb]
                        sqD = SQ.tile([128, SQ_SPLIT], BF16, tag="sqD")
                        sqA = SQ.tile([128, 2 * L - SQ_SPLIT], BF16, tag="sqA")
                        nc.vector.tensor_tensor(sqD[:, :], pt2[:, 0:SQ_SPLIT],
                                                pt2[:, 0:SQ_SPLIT], ALU.mult)
                        nc.scalar.activation(sqA[:, :], pt2[:, SQ_SPLIT:],
                                             AF.Square)
                        # N=17 matmuls: 16 bias cols + mean (wbc col16=1/128)
                        # into ONE psum bank [p, (ls2, kt, 17)]; E[x^2] via
                        # N=1 matmuls on the squared tiles into a stats bank.
                        # All stats then batch per-tile: 1 square, 1 subtract,
                        # 1 ln, 1 exp, 1 scale.
                        bB = PSB.tile([128, 2 * KT * 17], F32, tag="bB")
                        st = PSS.tile([128, 2 * KT], F32, tag="st")   # E2
                        for ls2 in range(2):
                            pT = pt2[:, ls2 * L:(ls2 + 1) * L]
                            for kt in range(KT):
                                nc.tensor.matmul(
                                    bB[:, (ls2 * KT + kt) * 17:
                                       (ls2 * KT + kt + 1) * 17],
                                    pT[:, kt * 128:(kt + 1) * 128],
                                    wbc_t[:, :], start=True, stop=True)
                            for kt in range(KT):
                                col = ls2 * L + kt * 128
                                sqsrc = (sqD[:, col:col + 128] if col < SQ_SPLIT
                                         else sqA[:, col - SQ_SPLIT:
                                                  col - SQ_SPLIT + 128])
                                nc.tensor.matmul(
                                    st[:, ls2 * KT + kt: ls2 * KT + kt + 1],
                                    sqsrc, wbc_t[:, 16:17], start=True, stop=True)
                        # m^2, layout [p, (ls2, kt)] matching st
                        m2 = SM.tile([128, 2 * KT], F32, tag="m2")
                        nc.scalar.activation(
                            m2[:, :].rearrange("p (ls2 kt) -> p ls2 kt", ls2=2),
                            bB[:, :].rearrange("p (ls2 kt c) -> p ls2 kt c",
                                               ls2=2, kt=KT)[:, :, :, 16],
                            AF.Square)
                        ve4 = SM.tile([128, 2 * KT], F32, tag="ve4")
                        nc.vector.tensor_tensor(ve4[:, :], st[:, :], m2[:, :],
                                                ALU.subtract)
                        # batched rsqrt via exp(-0.5*ln(x + eps))
                        lv4 = SM.tile([128, 2 * KT], F32, tag="lv4")
                        nc.scalar.activation(lv4[:, :], ve4[:, :], AF.Ln,
                                             bias=eps_c[:, :])
                        si4 = SM.tile([128, 2 * KT], F32, tag="si4")
                        nc.scalar.activation(si4[:, :], lv4[:, :], AF.Exp,
                                             scale=-0.5)
                        # scale: biasT[p, kt, lb*2+ls2, h] = bB[p, ls2, kt, 0:16]*si
                        y_ap = bB[:, :].rearrange("p (ls2 kt c) -> p kt ls2 c",
                                                  ls2=2, kt=KT)[:, :, :, 0:16]
                        s_ap = si4[:, :].rearrange("p (ls2 kt o) -> p kt ls2 o",
                                                   ls2=2, o=1).to_broadcast(
                                                       (128, KT, 2, H))
                        half, lloc = divmod(lb, 32)
                        b_ap = biasT_g[half][:, :].rearrange(
                            "p (kt l h) -> p kt l h", kt=KT,
                            l=LQ // 2)[:, :, lloc * 2:(lloc + 1) * 2, :]
                        nc.vector.tensor_tensor(b_ap, y_ap, s_ap, ALU.mult)

                    nb_tiles = LQ // 2 if "B" in phases else 0
                    for lb in range(min(A2_AT, nb_tiles)):
                        emit_b_tile(lb)
                    emit_a2()
                    for lb in range(min(A2_AT, nb_tiles), min(nb_tiles, 32)):
                        emit_b_tile(lb)

                # ---------------- Phase C: attention ------------------------
                # Emitted per l-HALF (64 query rows). Half 0's bias rows are
                # complete once B tile 31 lands, so its 16 heads interleave
                # with B tiles 32..63 — phase C's PE work runs under phase B's
                # DMA-paced stretch instead of serially after it. PSUM during
                # the interleave: B(3+2) + lg(2) + oLV(1) = 8 banks exactly;
                # the B pools close before the final-projection pool opens.
                LH = LQ // 2
                if True:
                    id_b = None
                    if "C" in phases:
                        id_b = CSM.tile([128, 128], BF16, tag="idb")
                        nc.vector.tensor_copy(id_b[:, :], id_t[:, :])

                    def emit_c_head(h, lh, PSL, PSO):
                        mc, i0 = h // 4, (h % 4) * 32
                        oLV = PSO.tile([LQ, 33], F32, tag="oLV")
                        oLVs = oLV[lh * LH:(lh + 1) * LH, :]
                        for kg in range(2):      # two [128, 4*LH] logit banks
                            lg = PSL.tile([128, 4 * LH], F32, tag="lg")
                            # Seed the bank with the bias in ONE identity
                            # matmul (start=True sets has_written — a DVE
                            # preload would be OVERWRITTEN by the first
                            # accumulating matmul on TRN2); kq accumulates.
                            b_ap = biasT_g[lh][:, kg * 4 * (LH * H):
                                               (kg + 1) * 4 * (LH * H)].rearrange(
                                "p (kt l h) -> p kt l h", kt=4, l=LH)[:, :, :, h]
                            nc.tensor.matmul(
                                lg[:, :].rearrange("p (kt l) -> p kt l", kt=4),
                                id_b[:, :], b_ap,
                                start=True, stop=False, skip_group_check=True)
                            for sub in range(4):
                                kt = kg * 4 + sub
                                sl = lg[:, sub * LH:(sub + 1) * LH]
                                nc.tensor.matmul(
                                    sl,
                                    kTb[i0:i0 + 32, mc * L + kt * 128: mc * L + (kt + 1) * 128],
                                    qTb[i0:i0 + 32, mc * LQ + lh * LH: mc * LQ + (lh + 1) * LH],
                                    start=False, stop=True, tile_position=(i0, 0),
                                    skip_group_check=True)
                            pr = PR.tile([128, 4 * LH], BF16, tag="pr")
                            if use_mask:
                                for sub in range(4):
                                    kt = kg * 4 + sub
                                    nc.scalar.activation(
                                        pr[:, sub * LH:(sub + 1) * LH],
                                        lg[:, sub * LH:(sub + 1) * LH], AF.Exp,
                                        bias=maskb_t[:, kt:kt + 1])
                            else:
                                nc.scalar.activation(pr[:, :], lg[:, :], AF.Exp)
                            for sub in range(4):
                                kt = kg * 4 + sub
                                nc.tensor.matmul(
                                    oLVs, pr[:, sub * LH:(sub + 1) * LH],
                                    v_sb[:, kt * (H * 33) + h * 33: kt * (H * 33) + (h + 1) * 33],
                                    start=(kt == 0), stop=(kt == KT - 1),
                                    skip_group_check=True)
                        dv_c = CSM.tile([LQ, 1], F32, tag="dv")
                        dv = dv_c[lh * LH:(lh + 1) * LH, :]
                        nc.vector.reciprocal(dv, oLVs[:, 32:33])
                        gd_c = CSM.tile([LQ, 1], F32, tag="gd")
                        gd = gd_c[lh * LH:(lh + 1) * LH, :]
                        nc.vector.tensor_tensor(
                            gd, gate[lh * LH:(lh + 1) * LH, h:h + 1], dv, ALU.mult)
                        nc.vector.tensor_scalar(
                            outN_g[h // 4][lh * LH:(lh + 1) * LH,
                                           (h % 4) * DH:(h % 4 + 1) * DH],
                            oLVs[:, 0:32], gd, None, ALU.mult)

                    interleave = "C" in phases and nb_tiles == LQ // 2
                    if interleave:
                        # half-0's pools: B(3+2) + lg(2) + oLV(1) = 8 banks
                        with (
                            tc.tile_pool(name="psL0", bufs=2, space="PSUM") as PSL0,
                            tc.tile_pool(name="psO0", bufs=1, space="PSUM") as PSO0,
                        ):
                            for k in range(H):
                                emit_c_head(k, 0, PSL0, PSO0)
                                emit_b_tile(32 + 2 * k)
                                emit_b_tile(33 + 2 * k)
                    else:
                        for lb in range(min(nb_tiles, 32), nb_tiles):
                            emit_b_tile(lb)
                    es_B.close()   # release phase B's 5 PSUM banks

                    with (
                        tc.tile_pool(name="psL1", bufs=4, space="PSUM") as PSL1,
                        tc.tile_pool(name="psO1", bufs=2, space="PSUM") as PSO1,
                        tc.tile_pool(name="psF", bufs=1, space="PSUM") as PSF,
                    ):
                        po = PSF.tile([LQ, D], F32, tag="po")

                        def emit_final_group(g):
                            # transpose this head-group's gated output and fold
                            # it into the Wo accumulation while later heads run
                            psT = PSF.tile([128, 4 * DH], F32, tag="psT")
                            nc.tensor.transpose(psT[:, :], outN_g[g][:, :],
                                                id_t[:, :])
                            nc.vector.tensor_copy(outg_g[g][:, :], psT[:, :])
                            nc.tensor.matmul(
                                po[:, :], outg_g[g][:, :],
                                wo_t[:, g * D:(g + 1) * D],
                                start=(g == 0), stop=(g == DC - 1),
                                skip_group_check=True)

                        if "C" in phases:
                            if not interleave:
                                for h in range(H):
                                    emit_c_head(h, 0, PSL1, PSO1)
                            for h in range(H):
                                emit_c_head(h, 1, PSL1, PSO1)
                                if h % 4 == 3:
                                    emit_final_group(h // 4)
                        else:
                            for g in range(DC):
                                nc.vector.memset(outN_g[g][:, :], 0.0)
                                emit_final_group(g)
                        nc.vector.tensor_copy(out_f[:, :], po[:, :])
                        # column-split the result DMA over 4 queues (one 256KB
                        # transfer would sit ~11us on a single queue at the
                        # very end of the kernel)
                        for j4 in range(DC):
                            nc.sync.dma_start(
                                out=out[:, j4 * 128:(j4 + 1) * 128],
                                in_=out_f[:, j4 * 128:(j4 + 1) * 128])
                    if DEBUG:
                        nc.sync.dma_start(out=d_gate[:, :], in_=gate[:, :])
                        nc.sync.dma_start(out=d_kTb[:, :], in_=kTb[:, :])
                        nc.sync.dma_start(out=d_qTb[:, :], in_=qTb[:, :])
                        for i in range(2):
                            nc.sync.dma_start(
                                out=d_biasT[:, i * KT * (LQ // 2) * H:
                                            (i + 1) * KT * (LQ // 2) * H],
                                in_=biasT_g[i][:, :])
                        for g in range(DC):
                            nc.sync.dma_start(out=d_outN[:, g * 128:(g + 1) * 128],
                                              in_=outN_g[g][:, :])
                        nc.sync.dma_start(out=d_vsb[:, :], in_=v_sb[:, :])
                    es_C.close()
    nc.compile()
    return nc


def _prep_inputs(single, pair, mask, ln_s_g, ln_s_b, Wq, bq, Wk, Wv,
                 ln_p_g, ln_p_b, Wb, Wg, Wo):
    f32 = np.float32
    single = np.asarray(single, f32).reshape(L, D)
    pair = np.asarray(pair, f32).reshape(L, L, P)
    maskv = np.asarray(mask).reshape(L).astype(bool)
    g_s = np.asarray(ln_s_g, f32); b_s = np.asarray(ln_s_b, f32)
    g_p = np.asarray(ln_p_g, f32)
    Wq = np.asarray(Wq, f32); Wk = np.asarray(Wk, f32); Wv = np.asarray(Wv, f32)
    Wg = np.asarray(Wg, f32); Wo = np.asarray(Wo, f32); Wb = np.asarray(Wb, f32)
    bq = np.asarray(bq, f32)

    sc = DH ** -0.5
    Wq2 = (g_s[:, None] * Wq) * sc
    bq2 = (b_s @ Wq + bq) * sc
    Wk2 = g_s[:, None] * Wk; bk2 = b_s @ Wk
    Wv2 = g_s[:, None] * Wv; bv2 = b_s @ Wv
    Wg2 = g_s[:, None] * Wg; bg2 = b_s @ Wg
    Wb2 = g_p[:, None] * Wb
    Wbc = Wb2 - Wb2.mean(0, keepdims=True)          # [128, 16]
    wbc_host = np.concatenate([Wbc, np.full((P, 1), 1.0 / P, f32)], axis=1)

    def pack_lhsT(W):   # [512, M] -> [128, 4*M] with (dc, mc-major cols)
        Din, M = W.shape
        return W.reshape(4, 128, M).transpose(1, 0, 2).reshape(128, 4 * M)

    bf = ml_dtypes.bfloat16
    wq_h = pack_lhsT(Wq2).astype(bf); wk_h = pack_lhsT(Wk2).astype(bf)
    wv_h = pack_lhsT(Wv2).astype(bf)
    wg_h = pack_lhsT(Wg2).astype(bf); wo_h = pack_lhsT(Wo).astype(bf)
    bq_h = bq2.reshape(4, 128).T.copy()
    bk_h = bk2.reshape(4, 128).T.copy()
    bv_h = np.broadcast_to(bv2, (128, D)).astype(bf)
    bgn_h = bg2.reshape(1, H).astype(bf)

    maskbias = np.where(maskv, 0.0, -1e9).astype(f32)
    pair_bf = pair.astype(ml_dtypes.bfloat16)

    in_maps = []
    for cid in range(NC):
        sh = -cid * LQ
        # Pre-transpose the core's pair slice to [lb][p][(ls2, k)] so the
        # device DMA is fully linear (4KB per partition row).
        sl = np.roll(pair_bf[cid * LQ:(cid + 1) * LQ], sh, axis=1)
        ptc = sl.transpose(2, 0, 1).reshape(128, LQ // 2, 2 * L)
        ptc = np.ascontiguousarray(ptc.transpose(1, 0, 2))
        in_maps.append({
            "pair_t": ptc,
            "single": np.roll(single, sh, axis=0).astype(ml_dtypes.bfloat16),
            "wq": wq_h, "wk": wk_h, "wv": wv_h, "wg": wg_h, "wo": wo_h,
            "wbc": wbc_host.astype(ml_dtypes.bfloat16),
            "bq": bq_h, "bk": bk_h, "bv": bv_h, "bgn": bgn_h,
            "maskb": np.roll(maskbias, sh).reshape(KT, 128).T.copy(),
            "ident": np.eye(128, dtype=f32),
            "out": np.zeros((LQ, D), f32),
            **({"d_gate": np.zeros((LQ, H), f32),
                "d_kTb": np.zeros((128, 4 * L), ml_dtypes.bfloat16),
                "d_qTb": np.zeros((128, 4 * LQ), ml_dtypes.bfloat16),
                "d_biasT": np.zeros((128, KT * LQ * H), ml_dtypes.bfloat16),
                "d_outN": np.zeros((LQ, D), f32),
                "d_vsb": np.zeros((128, KT * H * 33), ml_dtypes.bfloat16),
                "d_sT": np.zeros((128, 4 * L), ml_dtypes.bfloat16)} if DEBUG else {}),
        })
    return in_maps


def kernel(**inputs):
    use_mask = not np.asarray(inputs["mask"]).reshape(-1).astype(bool).all()
    key = ("nc", use_mask)
    if key not in _CACHED:
        _CACHED[key] = _build_bass(use_mask=use_mask)
    nc = _CACHED[key]
    in_maps = _prep_inputs(**inputs)
    res = run_bass_kernel_spmd(nc, in_maps, list(range(NC)),
                               trace=bool(LAST_INFO.get("want_trace")))
    LAST_INFO["results"] = res
    outs = [np.asarray(res.results[i]["out"]) for i in range(NC)]
    return np.concatenate(outs, axis=0).reshape(B, L, D).astype(np.float32)



# revision 62
# speedup vs baseline: 1.0194x; 1.0060x over previous
"""AttentionPairBias Trainium2 kernel (8 NeuronCores, query-sharded).

Strategy:
  - Shard the 1024 query rows across 8 cores (128 rows each). Each core reads
    only its slice of the huge pair tensor (512MB/8 = 64MB f32 -> 32MB bf16).
  - Host folds both LayerNorm affine transforms into the projection weights,
    centers the pair->bias weights so the pair-LN mean correction is free, and
    converts the pair slice to bf16 (DMA halved, enables DMA-transpose loads).
  - Per-head bias constant (ln_p_b @ Wb) is dropped: constant per (l,h) row is
    softmax-invariant.
  - On device, pair tiles arrive TRANSPOSED ([p, k] layout) via the DMA xbar,
    so the bias matmul contracts p on the PE with the pair tile as the
    stationary operand, producing [k, h] tiles; LN stats (mean / E[x^2]) come
    from extra matmul columns against ones/128; variance -> rsqrt is done as
    exp(-0.5*ln(var+eps)) so the whole kernel uses one ACT table set.
  - Attention runs transposed: logits^T[k,l] per (head, ktile), probs = exp()
    with the key-mask folded into the ACT bias operand, attn@v uses probs as
    the moving operand with a fused ones-column producing the softmax
    denominator for free. Output is built transposed, feeding the final Wo
    matmul without any extra transpose.
"""

import os

os.environ.setdefault("MYCRO_LOCAL_CACHE", "1")
# Tile's subtile dependency tracker mishandles interleaved strided APs (e.g.
# the [p, (dc, l)] transposed-activation writes) and lets consumers run before
# all producers; whole-tile deps are correct and cost nothing here since the
# kernel's phases are naturally sequential.
os.environ["BY_DEFAULT_DISABLE_SUBTILE_DEPS"] = "1"

import numpy as np
import ml_dtypes

# bass_utils imports antenv.axon_hooks unguarded when tracing is requested
# (e.g. BASS_TRACE=1 in the environment); some images lack that submodule.
# Provide the graceful no-hook fallback instead of an ImportError.
try:
    import antenv.axon_hooks  # noqa: F401
except ImportError:
    import sys as _sys
    import types as _types

    try:
        import antenv as _antenv
        _m = _types.ModuleType("antenv.axon_hooks")
        _hook = [None]
        _m.set_axon_ntff_profile_hook = lambda h: _hook.__setitem__(0, h)
        _m.get_axon_ntff_profile_hook = lambda: _hook[0]
        _sys.modules["antenv.axon_hooks"] = _m
        _antenv.axon_hooks = _m
    except ImportError:
        pass

# Prefer the ACT table set that contains Exp, Ln AND Square so the whole
# kernel needs exactly one table load. With the default set ordering the
# chooser alternates between an Exp/Square set and an Ln set inside the main
# loop, inserting ~270 table loads (~2.7us each).
import concourse.hw_specs as _hw_specs

_orig_get_act_tables = _hw_specs.get_activation_tables

def _patched_get_act_tables(arch):
    # Keep dict ORDER intact (set ids are positional — walrus loads tables by
    # index), but make natural_log_exp_and_others the only set offering Exp,
    # Ln and Square so every activation in this kernel resolves to one set.
    tabs = _orig_get_act_tables(arch)
    pref = "natural_log_exp_and_others"
    if pref not in tabs:
        return tabs
    strip = tabs[pref]
    return {
        k: (v if k == pref else (v - strip)) for k, v in tabs.items()
    }

_hw_specs.get_activation_tables = _patched_get_act_tables

import concourse.bass as bass
import concourse.bacc as bacc
import concourse.mybir as mybir
from concourse.bass_utils import run_bass_kernel_spmd
from concourse.tile import TileContext

F32 = mybir.dt.float32
F32R = mybir.dt.float32r
BF16 = mybir.dt.bfloat16
AF = mybir.ActivationFunctionType
ALU = mybir.AluOpType
AX = mybir.AxisListType

B, L, D, P, H = 1, 1024, 512, 128, 16
DH = D // H          # 32
NC = 8               # cores
LQ = L // NC         # 128 query rows per core
KT = L // 128        # 8 key tiles
DC = D // 128        # 4 D chunks
EPS = 1e-5

_CACHED = {}
LAST_INFO = {}
DEBUG = False


def _build_bass(phases="ABC", loop_n=None, use_mask=False):
    nc = bacc.Bacc("TRN2", target_bir_lowering=False, debug=False)
    # pair arrives HOST-pre-transposed: tile lb holds [p, (ls, k)] for the 4
    # query rows lb*4..lb*4+3 — a fully linear 1MB DMA (8KB per partition
    # row). The on-device DMA-transpose path ran at ~220GB/s and paced the
    # whole front half of the kernel; linear loads run at full HBM rate.
    pair_t = nc.declare_dram_parameter("pair_t", [LQ // 2, 128, 2 * L], BF16,
                                       isOutput=False)
    single = nc.declare_dram_parameter("single", [L, D], BF16, isOutput=False)
    wq = nc.declare_dram_parameter("wq", [128, 4 * D], BF16, isOutput=False)
    wk = nc.declare_dram_parameter("wk", [128, 4 * D], BF16, isOutput=False)
    wv = nc.declare_dram_parameter("wv", [128, 4 * D], BF16, isOutput=False)
    wg = nc.declare_dram_parameter("wg", [128, 4 * H], BF16, isOutput=False)
    wo = nc.declare_dram_parameter("wo", [128, 4 * D], BF16, isOutput=False)
    wbc = nc.declare_dram_parameter("wbc", [128, 17], BF16, isOutput=False)
    bq = nc.declare_dram_parameter("bq", [128, 4], F32, isOutput=False)
    bk = nc.declare_dram_parameter("bk", [128, 4], F32, isOutput=False)
    bv = nc.declare_dram_parameter("bv", [128, D], BF16, isOutput=False)
    bgn = nc.declare_dram_parameter("bgn", [1, H], BF16, isOutput=False)
    maskb = nc.declare_dram_parameter("maskb", [128, KT], F32, isOutput=False)
    ident = nc.declare_dram_parameter("ident", [128, 128], F32, isOutput=False)
    out = nc.declare_dram_parameter("out", [LQ, D], F32, isOutput=True)
    if DEBUG:
        d_gate = nc.declare_dram_parameter("d_gate", [LQ, H], F32, isOutput=True)
        d_kTb = nc.declare_dram_parameter("d_kTb", [128, 4 * L], BF16, isOutput=True)
        d_qTb = nc.declare_dram_parameter("d_qTb", [128, 4 * LQ], BF16, isOutput=True)
        d_biasT = nc.declare_dram_parameter("d_biasT", [128, KT * LQ * H], BF16, isOutput=True)
        d_outN = nc.declare_dram_parameter("d_outN", [LQ, D], F32, isOutput=True)
        d_vsb = nc.declare_dram_parameter("d_vsb", [128, KT * H * 33], BF16, isOutput=True)
        d_sT = nc.declare_dram_parameter("d_sT", [128, 4 * L], BF16, isOutput=True)

    with TileContext(nc) as tc:
        with tc.tile_pool(name="persist", bufs=1) as PS:
            kTb = PS.tile([128, 4 * L], BF16)        # [dk%128, (mc, k)]
            qTb = PS.tile([128, 4 * LQ], BF16)       # [dq%128, (mc, l)]
            v_sb = PS.tile([128, KT * (H * 33)], BF16)  # per kt: 16h x (32 v | 1 one)
            # bias, split per l-HALF so phase C's half-0 readers touch a
            # different tile than phase B's second-half writers (whole-tile
            # WAR edges would otherwise serialize the interleave)
            biasT_g = [PS.tile([128, KT * (LQ // 2) * H], BF16,
                               name=f"biasT{i}") for i in range(2)]
            gate = PS.tile([LQ, H], F32)
            wbc_t = PS.tile([128, 17], BF16)
            maskb_t = PS.tile([128, KT], F32)
            # weights split per-dc chunk: consumers read per-dc slices anyway,
            # and 4 separate 128KB DMAs spread across 4 queues instead of one
            # 512KB transfer camping on a single queue (~23us)
            wo_t = PS.tile([128, 4 * D], BF16)
            # gated attn out, split per 4-head group so the final transpose +
            # Wo accumulation can start as soon as its group's heads finish
            # (whole-tile deps would otherwise stall them to the very end)
            outN_g = [PS.tile([LQ, 4 * DH], F32, name=f"outN{dc}") for dc in range(DC)]
            outg_g = [PS.tile([128, LQ], BF16, name=f"outg{dc}") for dc in range(DC)]
            out_f = PS.tile([LQ, D], F32)
            id_t = PS.tile([128, 128], F32)
            eps_c = PS.tile([128, 1], F32)

            # Only the loads phase A1/B1 need immediately are issued here;
            # everything else is deferred behind the head-critical x + pair
            # triggers (each dma_start costs ~650ns of serial Sync-queue time,
            # so trigger ORDER sets the pipeline ramp).
            nc.sync.dma_start(out=id_t[:, :], in_=ident[:, :])
            nc.sync.dma_start(out=wbc_t[:, :], in_=wbc[:, :])
            nc.vector.memset(eps_c[:, :], EPS)
            import contextlib
            _loop_cm = tc.For_i(0, loop_n, 1) if loop_n else contextlib.nullcontext()
            with (
                _loop_cm,
                tc.tile_pool(name="pairp", bufs=16) as PP,
                tc.tile_pool(name="paw", bufs=1) as WW,
                tc.tile_pool(name="pax", bufs=8) as PX,
            ):
                # The first few pair tiles are issued BEFORE everything else:
                # their ~22us single-queue latency gates phase B's start, while
                # phase A tolerates its inputs arriving a few us later.
                npair = LQ // 2 if ("B" in phases or "D" in phases) else 0
                pt_tiles = []

                def emit_pair_dma(lb):
                    pt2 = PP.tile([128, 2 * L], BF16, tag="pt2")
                    nc.sync.dma_start(out=pt2[:, :], in_=pair_t[lb])
                    pt_tiles.append(pt2)

                # x tiles first (phase A starts off x[0]), then the pair head
                x_tiles = []
                for lt in range(L // 128 if "A" in phases else 0):
                    x = PX.tile([128, D], BF16, tag="x")
                    nc.sync.dma_start(out=x[:, :], in_=single[lt * 128:(lt + 1) * 128, :])
                    x_tiles.append(x)
                NPRE = 6
                for lb in range(min(NPRE, npair)):
                    emit_pair_dma(lb)
                # weights: not needed until A2 / phase C
                wq_t = WW.tile([128, 4 * D], BF16)
                wk_t = WW.tile([128, 4 * D], BF16)
                wv_t = WW.tile([128, 4 * D], BF16)
                wg_t = WW.tile([128, 4 * H], BF16)
                bq_t = WW.tile([128, 4], F32)
                bk_t = WW.tile([128, 4], F32)
                bv_t = WW.tile([128, D], BF16)
                bg_t = WW.tile([1, H], BF16)
                ones_t = WW.tile([1, LQ], BF16)
                nc.sync.dma_start(out=wq_t[:, :], in_=wq[:, :])
                nc.sync.dma_start(out=wk_t[:, :], in_=wk[:, :])
                nc.sync.dma_start(out=wv_t[:, :], in_=wv[:, :])
                nc.sync.dma_start(out=wg_t[:, :], in_=wg[:, :])
                nc.sync.dma_start(out=bq_t[:, :], in_=bq[:, :])
                nc.sync.dma_start(out=bk_t[:, :], in_=bk[:, :])
                nc.sync.dma_start(out=bv_t[:, :], in_=bv[:, :])
                nc.sync.dma_start(out=bg_t[:, :], in_=bgn[:, :])
                nc.sync.dma_start(out=maskb_t[:, :], in_=maskb[:, :])
                nc.sync.dma_start(out=wo_t[:, :], in_=wo[:, :])
                nc.vector.memset(ones_t[:, :], 1.0)
                # Remaining pair loads (pre-transposed on host). Each dma_start
                # lands on ONE of the 16 queues (~22GB/s each), so tile size
                # sets the latency-to-first-tile: half-size 512KB tiles (2
                # query rows) arrive in ~22us, and a 16-deep pool covers the
                # bandwidth-delay product so the stream never starves.
                # (Partition-split sub-DMAs are NOT used: <128-partition
                # transfers lose AXI ports to the swizzle and run ~1.6x
                # slower. Compute-queue issue is also out: a pool-gated DMA
                # trigger would stall all compute behind it.)
                for lb in range(NPRE, npair):
                    emit_pair_dma(lb)

                if "D" in phases and "B" not in phases:
                    with tc.tile_pool(name="dsink", bufs=2) as DS:
                        for lb in range(LQ // 2):
                            snk = DS.tile([128, 1], BF16, tag="snk")
                            nc.vector.tensor_copy(snk[:, :], pt_tiles[lb][:, 0:1])
                # ---------------- Phase A1: LN(single) -> sT -----------------
                # Engine queues are in-order, so phase A's PE instructions
                # ahead of phase B's would delay B ~30us past its data. Emit
                # A1 (cheap transposes), then the first B tiles, then A2 (the
                # projection matmuls) under B's engine slack, then the rest
                # of B.
                sT = PS.tile([128, 4 * L], BF16)  # xhat^T: [D%128, (dc, l)]
                with (
                    tc.tile_pool(name="pa", bufs=2) as PA,
                    tc.tile_pool(name="pasm", bufs=3) as SM,
                    tc.tile_pool(name="paps", bufs=2, space="PSUM") as PSA,
                ):
                    for lt in range(L // 128 if "A" in phases else 0):
                        x = x_tiles[lt]          # bf16 (upconverted on read)
                        S = SM.tile([128, 1], F32, tag="S")
                        nc.vector.tensor_reduce(S[:, :], x[:, :], AX.X, ALU.add)
                        sq = PA.tile([128, D], F32, tag="sq")
                        Q = SM.tile([128, 1], F32, tag="Q")
                        nc.scalar.activation(sq[:, :], x[:, :], AF.Square, accum_out=Q[:, :])
                        m = SM.tile([128, 1], F32, tag="m")
                        nc.vector.tensor_scalar(m[:, :], S[:, :], 1.0 / D, None, ALU.mult)
                        m2 = SM.tile([128, 1], F32, tag="m2")
                        nc.vector.tensor_tensor(m2[:, :], m[:, :], m[:, :], ALU.mult)
                        q2 = SM.tile([128, 1], F32, tag="q2")
                        nc.vector.tensor_scalar(q2[:, :], Q[:, :], 1.0 / D, EPS, ALU.mult, ALU.add)
                        ve = SM.tile([128, 1], F32, tag="ve")
                        nc.vector.tensor_tensor(ve[:, :], q2[:, :], m2[:, :], ALU.subtract)
                        lv = SM.tile([128, 1], F32, tag="lv")
                        nc.scalar.activation(lv[:, :], ve[:, :], AF.Ln)
                        sinv = SM.tile([128, 1], F32, tag="si")
                        nc.scalar.activation(sinv[:, :], lv[:, :], AF.Exp, scale=-0.5)
                        xh = PA.tile([128, D], F32, tag="xh")
                        nc.vector.tensor_scalar(xh[:, :], x[:, :], m[:, :], sinv[:, :],
                                                ALU.subtract, ALU.mult)
                        pst = PSA.tile([128, D], F32, tag="pst")
                        for j in range(DC):
                            nc.tensor.transpose(pst[:, j * 128:(j + 1) * 128],
                                                xh[:, j * 128:(j + 1) * 128], id_t[:, :])
                        o_ap = sT[:, :].rearrange("p (dc n) -> p dc n", dc=DC)[:, :, lt * 128:(lt + 1) * 128]
                        i_ap = pst[:, :].rearrange("p (dc j) -> p dc j", dc=DC)
                        nc.vector.tensor_copy(o_ap, i_ap)

                def emit_a2():
                    # ------------ Phase A2: projections off sT ---------------
                    with (
                        tc.tile_pool(name="pa2sm", bufs=3) as SM,
                        tc.tile_pool(name="paps2", bufs=1, space="PSUM") as PSA,
                    ):
                        # kT (keys, transposed, bf16)
                        for mc in range(4 if "A" in phases else 0):
                            for nb in range(2):
                                ps = PSA.tile([128, 512], F32, tag="kv")
                                for dc in range(DC):
                                    nc.tensor.matmul(
                                        ps[:, :],
                                        wk_t[:, dc * D + mc * 128: dc * D + (mc + 1) * 128],
                                        sT[:, dc * L + nb * 512: dc * L + (nb + 1) * 512],
                                        start=(dc == 0), stop=(dc == DC - 1))
                                nc.vector.tensor_scalar(
                                    kTb[:, mc * L + nb * 512: mc * L + (nb + 1) * 512],
                                    ps[:, :], bk_t[:, mc:mc + 1], None, ALU.add)
                        # v (natural layout, h-interleaved with ones column)
                        for kt in range(KT if "A" in phases else 0):
                            ps = PSA.tile([128, 512], F32, tag="kv")
                            for dc in range(DC):
                                nc.tensor.matmul(
                                    ps[:, :],
                                    sT[:, dc * L + kt * 128: dc * L + (kt + 1) * 128],
                                    wv_t[:, dc * D:(dc + 1) * D],
                                    start=(dc == 0), stop=(dc == DC - 1))
                            o_ap = v_sb[:, kt * (H * 33):(kt + 1) * (H * 33)].rearrange(
                                "p (h x) -> p h x", h=H)[:, :, 0:32]
                            nc.vector.tensor_tensor(
                                o_ap, ps[:, :].rearrange("p (h x) -> p h x", h=H),
                                bv_t[:, :].rearrange("p (h x) -> p h x", h=H), ALU.add)
                        # qT for own 128 rows (device rows 0..127 via host roll)
                        for mc in range(4 if "A" in phases else 0):
                            ps = PSA.tile([128, LQ], F32, tag="q")
                            for dc in range(DC):
                                nc.tensor.matmul(
                                    ps[:, :],
                                    wq_t[:, dc * D + mc * 128: dc * D + (mc + 1) * 128],
                                    sT[:, dc * L: dc * L + LQ],
                                    start=(dc == 0), stop=(dc == DC - 1))
                            nc.vector.tensor_scalar(
                                qTb[:, mc * LQ:(mc + 1) * LQ], ps[:, :],
                                bq_t[:, mc:mc + 1], None, ALU.add)
                        # gate = sigmoid(xhat @ Wg + bg) = 1/(1+exp(-x-bg))
                        if "A" not in phases:
                            nc.vector.memset(gate[:, :], 0.5)
                        psg = PSA.tile([LQ, H], F32, tag="g", name="psg") if "A" in phases else None
                        for dc in range(DC if "A" in phases else 0):
                            nc.tensor.matmul(
                                psg[:, :],
                                sT[:, dc * L: dc * L + LQ],
                                wg_t[:, dc * H:(dc + 1) * H],
                                start=(dc == 0), stop=False)
                        if "A" in phases:
                            nc.tensor.matmul(psg[:, :], ones_t[:, :], bg_t[:, :],
                                             start=False, stop=True)
                        if "A" in phases:
                            eg = SM.tile([LQ, H], F32, tag="eg")
                            nc.scalar.activation(eg[:, :], psg[:, :], AF.Exp, scale=-1.0)
                            eg1 = SM.tile([LQ, H], F32, tag="eg1")
                            nc.vector.tensor_scalar(eg1[:, :], eg[:, :], 1.0, None, ALU.add)
                            nc.vector.reciprocal(gate[:, :], eg1[:, :])
                        # ones column of v_sb
                        ones_ap = v_sb[:, :].rearrange("p (kt h x) -> p kt h x", kt=KT, h=H)[:, :, :, 32:33]
                        nc.vector.memset(ones_ap, 1.0)
                        if DEBUG:
                            nc.sync.dma_start(out=d_sT[:, :], in_=sT[:, :])

                # ---------------- Phase B: pair -> biasT ---------------------
                if "B" not in phases:
                    nc.vector.memset(biasT_g[0][:, :], 0.0)
                    nc.vector.memset(biasT_g[1][:, :], 0.0)
                A2_AT = 9   # emit A2's matmuls after this many B tiles
                import contextlib as _ctxlib
                # Phase C's pools are OPENED first (bank reservation is lazy,
                # at first tile alloc) so the B pools sit on top of the pool
                # stack and can be released mid-emission, LIFO-correctly,
                # before the final-projection pool opens.
                es_C = _ctxlib.ExitStack()
                CSM = es_C.enter_context(tc.tile_pool(name="pcsm", bufs=4))
                PR = es_C.enter_context(tc.tile_pool(name="probs", bufs=8))
                es_B = _ctxlib.ExitStack()
                SQ = es_B.enter_context(tc.tile_pool(name="sqp", bufs=3))
                SM = es_B.enter_context(tc.tile_pool(name="pbsm", bufs=12))
                PSB = es_B.enter_context(
                    tc.tile_pool(name="psB", bufs=3, space="PSUM"))
                PSS = es_B.enter_context(
                    tc.tile_pool(name="psS", bufs=2, space="PSUM"))
                if True:
                    # DVE squares [0:SPLIT], ACT squares the rest — into TWO
                    # separate tiles: with subtile deps disabled, two engines
                    # writing halves of one tile get WAW-serialized and the
                    # whole per-tile chain stops pipelining.
                    SQ_SPLIT = 1280

                    def emit_b_tile(lb):
                        pt2 = pt_tiles[lb]
                        sqD = SQ.tile([128, SQ_SPLIT], BF16, tag="sqD")
                        sqA = SQ.tile([128, 2 * L - SQ_SPLIT], BF16, tag="sqA")
                        nc.vector.tensor_tensor(sqD[:, :], pt2[:, 0:SQ_SPLIT],
                                                pt2[:, 0:SQ_SPLIT], ALU.mult)
                        nc.scalar.activation(sqA[:, :], pt2[:, SQ_SPLIT:],
                                             AF.Square)
                        # N=17 matmuls: 16 bias cols + mean (wbc col16=1/128)
                        # into ONE psum bank [p, (ls2, kt, 17)]; E[x^2] via
                        # N=1 matmuls on the squared tiles into a stats bank.
                        # All stats then batch per-tile: 1 square, 1 subtract,
                        # 1 ln, 1 exp, 1 scale.
                        bB = PSB.tile([128, 2 * KT * 17], F32, tag="bB")
                        st = PSS.tile([128, 2 * KT], F32, tag="st")   # E2
                        for ls2 in range(2):
                            pT = pt2[:, ls2 * L:(ls2 + 1) * L]
                            for kt in range(KT):
                                nc.tensor.matmul(
                                    bB[:, (ls2 * KT + kt) * 17:
                                       (ls2 * KT + kt + 1) * 17],
                                    pT[:, kt * 128:(kt + 1) * 128],
                                    wbc_t[:, :], start=True, stop=True)
                            for kt in range(KT):
                                col = ls2 * L + kt * 128
                                sqsrc = (sqD[:, col:col + 128] if col < SQ_SPLIT
                                         else sqA[:, col - SQ_SPLIT:
                                                  col - SQ_SPLIT + 128])
                                nc.tensor.matmul(
                                    st[:, ls2 * KT + kt: ls2 * KT + kt + 1],
                                    sqsrc, wbc_t[:, 16:17], start=True, stop=True)
                        # m^2, layout [p, (ls2, kt)] matching st
                        m2 = SM.tile([128, 2 * KT], F32, tag="m2")
                        nc.scalar.activation(
                            m2[:, :].rearrange("p (ls2 kt) -> p ls2 kt", ls2=2),
                            bB[:, :].rearrange("p (ls2 kt c) -> p ls2 kt c",
                                               ls2=2, kt=KT)[:, :, :, 16],
                            AF.Square)
                        ve4 = SM.tile([128, 2 * KT], F32, tag="ve4")
                        nc.vector.tensor_tensor(ve4[:, :], st[:, :], m2[:, :],
                                                ALU.subtract)
                        # batched rsqrt via exp(-0.5*ln(x + eps))
                        lv4 = SM.tile([128, 2 * KT], F32, tag="lv4")
                        nc.scalar.activation(lv4[:, :], ve4[:, :], AF.Ln,
                                             bias=eps_c[:, :])
                        si4 = SM.tile([128, 2 * KT], F32, tag="si4")
                        nc.scalar.activation(si4[:, :], lv4[:, :], AF.Exp,
                                             scale=-0.5)
                        # scale: biasT[p, kt, lb*2+ls2, h] = bB[p, ls2, kt, 0:16]*si
                        y_ap = bB[:, :].rearrange("p (ls2 kt c) -> p kt ls2 c",
                                                  ls2=2, kt=KT)[:, :, :, 0:16]
                        s_ap = si4[:, :].rearrange("p (ls2 kt o) -> p kt ls2 o",
                                                   ls2=2, o=1).to_broadcast(
                                                       (128, KT, 2, H))
                        half, lloc = divmod(lb, 32)
                        b_ap = biasT_g[half][:, :].rearrange(
                            "p (kt l h) -> p kt l h", kt=KT,
                            l=LQ // 2)[:, :, lloc * 2:(lloc + 1) * 2, :]
                        nc.vector.tensor_tensor(b_ap, y_ap, s_ap, ALU.mult)

                    nb_tiles = LQ // 2 if "B" in phases else 0
                    for lb in range(min(A2_AT, nb_tiles)):
                        emit_b_tile(lb)
                    emit_a2()
                    for lb in range(min(A2_AT, nb_tiles), min(nb_tiles, 32)):
                        emit_b_tile(lb)

                # ---------------- Phase C: attention ------------------------
                # Emitted per l-HALF (64 query rows). Half 0's bias rows are
                # complete once B tile 31 lands, so its 16 heads interleave
                # with B tiles 32..63 — phase C's PE work runs under phase B's
                # DMA-paced stretch instead of serially after it. PSUM during
                # the interleave: B(3+2) + lg(2) + oLV(1) = 8 banks exactly;
                # the B pools close before the final-projection pool opens.
                LH = LQ // 2
                if True:
                    id_b = None
                    if "C" in phases:
                        id_b = CSM.tile([128, 128], BF16, tag="idb")
                        nc.vector.tensor_copy(id_b[:, :], id_t[:, :])

                    def emit_c_head(h, lh, PSL, PSO):
                        mc, i0 = h // 4, (h % 4) * 32
                        oLV = PSO.tile([LQ, 33], F32, tag="oLV")
                        oLVs = oLV[lh * LH:(lh + 1) * LH, :]
                        for kg in range(2):      # two [128, 4*LH] logit banks
                            lg = PSL.tile([128, 4 * LH], F32, tag="lg")
                            # Seed the bank with the bias in ONE identity
                            # matmul (start=True sets has_written — a DVE
                            # preload would be OVERWRITTEN by the first
                            # accumulating matmul on TRN2); kq accumulates.
                            b_ap = biasT_g[lh][:, kg * 4 * (LH * H):
                                               (kg + 1) * 4 * (LH * H)].rearrange(
                                "p (kt l h) -> p kt l h", kt=4, l=LH)[:, :, :, h]
                            nc.tensor.matmul(
                                lg[:, :].rearrange("p (kt l) -> p kt l", kt=4),
                                id_b[:, :], b_ap,
                                start=True, stop=False, skip_group_check=True)
                            for sub in range(4):
                                kt = kg * 4 + sub
                                sl = lg[:, sub * LH:(sub + 1) * LH]
                                nc.tensor.matmul(
                                    sl,
                                    kTb[i0:i0 + 32, mc * L + kt * 128: mc * L + (kt + 1) * 128],
                                    qTb[i0:i0 + 32, mc * LQ + lh * LH: mc * LQ + (lh + 1) * LH],
                                    start=False, stop=True, tile_position=(i0, 0),
                                    skip_group_check=True)
                            pr = PR.tile([128, 4 * LH], BF16, tag="pr")
                            if use_mask:
                                for sub in range(4):
                                    kt = kg * 4 + sub
                                    nc.scalar.activation(
                                        pr[:, sub * LH:(sub + 1) * LH],
                                        lg[:, sub * LH:(sub + 1) * LH], AF.Exp,
                                        bias=maskb_t[:, kt:kt + 1])
                            else:
                                nc.scalar.activation(pr[:, :], lg[:, :], AF.Exp)
                            for sub in range(4):
                                kt = kg * 4 + sub
                                nc.tensor.matmul(
                                    oLVs, pr[:, sub * LH:(sub + 1) * LH],
                                    v_sb[:, kt * (H * 33) + h * 33: kt * (H * 33) + (h + 1) * 33],
                                    start=(kt == 0), stop=(kt == KT - 1),
                                    skip_group_check=True)
                        dv_c = CSM.tile([LQ, 1], F32, tag="dv")
                        dv = dv_c[lh * LH:(lh + 1) * LH, :]
                        nc.vector.reciprocal(dv, oLVs[:, 32:33])
                        gd_c = CSM.tile([LQ, 1], F32, tag="gd")
                        gd = gd_c[lh * LH:(lh + 1) * LH, :]
                        nc.vector.tensor_tensor(
                            gd, gate[lh * LH:(lh + 1) * LH, h:h + 1], dv, ALU.mult)
                        nc.vector.tensor_scalar(
                            outN_g[h // 4][lh * LH:(lh + 1) * LH,
                                           (h % 4) * DH:(h % 4 + 1) * DH],
                            oLVs[:, 0:32], gd, None, ALU.mult)

                    interleave = "C" in phases and nb_tiles == LQ // 2
                    if interleave:
                        # half-0's pools: B(3+2) + lg(2) + oLV(1) = 8 banks
                        with (
                            tc.tile_pool(name="psL0", bufs=2, space="PSUM") as PSL0,
                            tc.tile_pool(name="psO0", bufs=1, space="PSUM") as PSO0,
                        ):
                            # coarse chunks: 4 heads per 8 B tiles — finer
                            # interleaving thrashes the in-order queues (every
                            # cross-engine wait in a C head blocks the B work
                            # queued behind it)
                            for chunk in range(4):
                                for hh in range(4):
                                    emit_c_head(chunk * 4 + hh, 0, PSL0, PSO0)
                                for t in range(8):
                                    emit_b_tile(32 + chunk * 8 + t)
                    else:
                        for lb in range(min(nb_tiles, 32), nb_tiles):
                            emit_b_tile(lb)
                    es_B.close()   # release phase B's 5 PSUM banks

                    with (
                        tc.tile_pool(name="psL1", bufs=4, space="PSUM") as PSL1,
                        tc.tile_pool(name="psO1", bufs=2, space="PSUM") as PSO1,
                        tc.tile_pool(name="psF", bufs=1, space="PSUM") as PSF,
                    ):
                        po = PSF.tile([LQ, D], F32, tag="po")

                        def emit_final_group(g):
                            # transpose this head-group's gated output and fold
                            # it into the Wo accumulation while later heads run
                            psT = PSF.tile([128, 4 * DH], F32, tag="psT")
                            nc.tensor.transpose(psT[:, :], outN_g[g][:, :],
                                                id_t[:, :])
                            nc.vector.tensor_copy(outg_g[g][:, :], psT[:, :])
                            nc.tensor.matmul(
                                po[:, :], outg_g[g][:, :],
                                wo_t[:, g * D:(g + 1) * D],
                                start=(g == 0), stop=(g == DC - 1),
                                skip_group_check=True)

                        if "C" in phases:
                            if not interleave:
                                for h in range(H):
                                    emit_c_head(h, 0, PSL1, PSO1)
                            for h in range(H):
                                emit_c_head(h, 1, PSL1, PSO1)
                                if h % 4 == 3:
                                    emit_final_group(h // 4)
                        else:
                            for g in range(DC):
                                nc.vector.memset(outN_g[g][:, :], 0.0)
                                emit_final_group(g)
                        nc.vector.tensor_copy(out_f[:, :], po[:, :])
                        # column-split the result DMA over 4 queues (one 256KB
                        # transfer would sit ~11us on a single queue at the
                        # very end of the kernel)
                        for j4 in range(DC):
                            nc.sync.dma_start(
                                out=out[:, j4 * 128:(j4 + 1) * 128],
                                in_=out_f[:, j4 * 128:(j4 + 1) * 128])
                    if DEBUG:
                        nc.sync.dma_start(out=d_gate[:, :], in_=gate[:, :])
                        nc.sync.dma_start(out=d_kTb[:, :], in_=kTb[:, :])
                        nc.sync.dma_start(out=d_qTb[:, :], in_=qTb[:, :])
                        for i in range(2):
                            nc.sync.dma_start(
                                out=d_biasT[:, i * KT * (LQ // 2) * H:
                                            (i + 1) * KT * (LQ // 2) * H],
                                in_=biasT_g[i][:, :])
                        for g in range(DC):
                            nc.sync.dma_start(out=d_outN[:, g * 128:(g + 1) * 128],
                                              in_=outN_g[g][:, :])
                        nc.sync.dma_start(out=d_vsb[:, :], in_=v_sb[:, :])
                    es_C.close()
    nc.compile()
    return nc


def _prep_inputs(single, pair, mask, ln_s_g, ln_s_b, Wq, bq, Wk, Wv,
                 ln_p_g, ln_p_b, Wb, Wg, Wo):
    f32 = np.float32
    single = np.asarray(single, f32).reshape(L, D)
    pair = np.asarray(pair, f32).reshape(L, L, P)
    maskv = np.asarray(mask).reshape(L).astype(bool)
    g_s = np.asarray(ln_s_g, f32); b_s = np.asarray(ln_s_b, f32)
    g_p = np.asarray(ln_p_g, f32)
    Wq = np.asarray(Wq, f32); Wk = np.asarray(Wk, f32); Wv = np.asarray(Wv, f32)
    Wg = np.asarray(Wg, f32); Wo = np.asarray(Wo, f32); Wb = np.asarray(Wb, f32)
    bq = np.asarray(bq, f32)

    sc = DH ** -0.5
    Wq2 = (g_s[:, None] * Wq) * sc
    bq2 = (b_s @ Wq + bq) * sc
    Wk2 = g_s[:, None] * Wk; bk2 = b_s @ Wk
    Wv2 = g_s[:, None] * Wv; bv2 = b_s @ Wv
    Wg2 = g_s[:, None] * Wg; bg2 = b_s @ Wg
    Wb2 = g_p[:, None] * Wb
    Wbc = Wb2 - Wb2.mean(0, keepdims=True)          # [128, 16]
    wbc_host = np.concatenate([Wbc, np.full((P, 1), 1.0 / P, f32)], axis=1)

    def pack_lhsT(W):   # [512, M] -> [128, 4*M] with (dc, mc-major cols)
        Din, M = W.shape
        return W.reshape(4, 128, M).transpose(1, 0, 2).reshape(128, 4 * M)

    bf = ml_dtypes.bfloat16
    wq_h = pack_lhsT(Wq2).astype(bf); wk_h = pack_lhsT(Wk2).astype(bf)
    wv_h = pack_lhsT(Wv2).astype(bf)
    wg_h = pack_lhsT(Wg2).astype(bf); wo_h = pack_lhsT(Wo).astype(bf)
    bq_h = bq2.reshape(4, 128).T.copy()
    bk_h = bk2.reshape(4, 128).T.copy()
    bv_h = np.broadcast_to(bv2, (128, D)).astype(bf)
    bgn_h = bg2.reshape(1, H).astype(bf)

    maskbias = np.where(maskv, 0.0, -1e9).astype(f32)
    pair_bf = pair.astype(ml_dtypes.bfloat16)

    in_maps = []
    for cid in range(NC):
        sh = -cid * LQ
        # Pre-transpose the core's pair slice to [lb][p][(ls2, k)] so the
        # device DMA is fully linear (4KB per partition row).
        sl = np.roll(pair_bf[cid * LQ:(cid + 1) * LQ], sh, axis=1)
        ptc = sl.transpose(2, 0, 1).reshape(128, LQ // 2, 2 * L)
        ptc = np.ascontiguousarray(ptc.transpose(1, 0, 2))
        in_maps.append({
            "pair_t": ptc,
            "single": np.roll(single, sh, axis=0).astype(ml_dtypes.bfloat16),
            "wq": wq_h, "wk": wk_h, "wv": wv_h, "wg": wg_h, "wo": wo_h,
            "wbc": wbc_host.astype(ml_dtypes.bfloat16),
            "bq": bq_h, "bk": bk_h, "bv": bv_h, "bgn": bgn_h,
            "maskb": np.roll(maskbias, sh).reshape(KT, 128).T.copy(),
            "ident": np.eye(128, dtype=f32),
            "out": np.zeros((LQ, D), f32),
            **({"d_gate": np.zeros((LQ, H), f32),
                "d_kTb": np.zeros((128, 4 * L), ml_dtypes.bfloat16),
                "d_qTb": np.zeros((128, 4 * LQ), ml_dtypes.bfloat16),
                "d_biasT": np.zeros((128, KT * LQ * H), ml_dtypes.bfloat16),
                "d_outN": np.zeros((LQ, D), f32),
                "d_vsb": np.zeros((128, KT * H * 33), ml_dtypes.bfloat16),
                "d_sT": np.zeros((128, 4 * L), ml_dtypes.bfloat16)} if DEBUG else {}),
        })
    return in_maps


def kernel(**inputs):
    use_mask = not np.asarray(inputs["mask"]).reshape(-1).astype(bool).all()
    key = ("nc", use_mask)
    if key not in _CACHED:
        _CACHED[key] = _build_bass(use_mask=use_mask)
    nc = _CACHED[key]
    in_maps = _prep_inputs(**inputs)
    res = run_bass_kernel_spmd(nc, in_maps, list(range(NC)),
                               trace=bool(LAST_INFO.get("want_trace")))
    LAST_INFO["results"] = res
    outs = [np.asarray(res.results[i]["out"]) for i in range(NC)]
    return np.concatenate(outs, axis=0).reshape(B, L, D).astype(np.float32)

